# revision 13
# baseline (speedup 1.0000x reference)
"""Trainium2 Bass kernel: 2-layer GAT (nn_GAT_1709396983866), v3.

Strategy (graph/data parallel over 8 NeuronCores):
  * Nodes are permuted and packed into 784 blocks of 128 positions, balanced
    by in-degree (~2041 edges/block). Core k owns blocks [98k, 98k+98); edges
    are sharded by destination block so segment-softmax / scatter-add stay
    core-local. Each block's edges form chunks of 128 (same dst block); the
    per-slot chunk counts are padded to a cross-core-uniform grid `cm` so all
    8 cores run one SPMD program.
  * Per-edge feature gathers use multi-offset `indirect_dma_start`: one call
    gathers ~28 chunks' worth of rows (int32 offsets, [128, k] offset AP)
    instead of v1's one call per chunk — the ~1us SWDGE fixed cost (the v1
    bottleneck: 6272 calls x 1.09us serialized on GpSimd) is amortized ~28x.
  * Gather tables:
      T1    [npad, 136] bf16: [asrc1(8) | h(128)]      (by src, layer 1)
      DSTT  [npad, 8]   bf16: adst1                    (by dst, layer 1)
      T2    [npad, 41]  f32:  [asrc2(1) | h2(40)]      (by src, layer 2)
      DSTT2 [12544, 1]  f32:  adst2       (core-local, by dst, layer 2)
    T1/DSTT are computed replicated in phase A; T2 shards are AllGathered.
  * Per chunk: one-hot mask[e,d] = (dst_local[e]==d); logits = asrc[src] +
    adst[dst] from the two gathers; exp(leaky_relu(x)) = max(exp(x),
    exp(0.2x)) (exact, by monotonicity); messages [h*exp | exp] are
    scatter-added per dst block via mask.T @ msg on the tensor engine (PSUM
    accumulation, one bank per block slot so start=True bank-clears can't
    clobber a sibling accumulation).
  * Blocks are processed in groups of 7, chunks in ranges of ~28; all
    elementwise work is batched per range (~3600 elems/partition per DVE
    instruction) to amortize the ~150-290ns per-instruction overheads.

kernel(**inputs) takes the full unsharded inputs and returns the full output.
"""

import numpy as np
import ml_dtypes

import concourse.bass as bass
import concourse.tile as tile
from concourse import mybir
from concourse.bass_utils import run_bass_kernel_spmd
from concourse.tile_rust import add_dep_helper


# Per-opcode embedded sync-wait slot budget in walrus codegen (empirical).
_WAIT_LIMITS = {}
_WAIT_LIMIT_DEFAULT = 1
_NOSPLIT_OPS = ("EventSemaphore",)


def _split_excess_waits(nc):
    """Move excess sem waits onto preceding same-engine wait instructions."""
    nid = [0]

    def mk_wait(engine, wait):
        nid[0] += 1
        ev = mybir.InstEventSemaphore(
            name=f"waitsplit-{nid[0]}", ins=[], outs=[])
        ev.engine = engine
        ev.sync_info = mybir.SyncInfo(on_wait=[wait], on_update=[])
        return ev

    for fn in nc.m.functions:
        for bb in fn.blocks:
            out = []
            for inst in bb.instructions:
                si = inst.sync_info
                waits = list(si.on_wait) if si and si.on_wait else []
                lim = _WAIT_LIMITS.get(inst.opcode, _WAIT_LIMIT_DEFAULT)
                if len(waits) > lim and inst.opcode not in _NOSPLIT_OPS:
                    excess, keep = waits[:-lim], waits[-lim:]
                    for w in excess:
                        out.append(mk_wait(inst.engine, w))
                    inst.sync_info = mybir.SyncInfo(
                        on_wait=keep, on_update=list(si.on_update or []))
                out.append(inst)
            bb.instructions = out


def _phase_barrier(tc, nc):
    """All-engine barrier that soaks per-DMA-lane waits per engine first."""
    curr_bb = nc.cur_bb
    prev = list(curr_bb.bb.instructions)
    for eng in (nc.gpsimd, nc.sync, nc.scalar, nc.vector, nc.tensor):
        nop = eng.nop()
        for inst in prev:
            add_dep_helper(
                nop.ins, inst,
                sync=bass.sync_unless_reorderable_target(
                    inst, inst.is_executable()),
                reason="phase-barrier soak")
    tc.strict_bb_all_engine_barrier()


# -------- problem constants (hardcoded, per spec) --------
N_NODES = 100000
IN_DIM = 128
HID = 128
OUT_DIM = 40
H1 = 8
C1 = 16
NEG_SLOPE = 0.2
EPS = 1e-16
DENOM_FLOOR = 1e-6
N_CORES = 8
P = 128
NBLK = 784
BPC = NBLK // N_CORES          # 98
NPAD = NBLK * P                # 100352
G = 7                          # block slots per group
NGRP = BPC // G                # 14
GROWS = G * P                  # 896 rows per group
PADLOC = 200.0                 # dst_local for padding edge slots
TPB = 4                        # blocks per phase-A batch
NRQ = 4                        # chunk ranges per group (gather call batches)

T1W = H1 + HID                 # 136: [asrc1 | h]
DSTW = H1                      # 8: adst1
T2W = 1 + OUT_DIM              # 41: [asrc2 | h2]
ACC1W = HID + H1               # 136
ACC2W = OUT_DIM + 1            # 41

F32 = mybir.dt.float32
BF16 = mybir.dt.bfloat16
I32 = mybir.dt.int32
AF = mybir.ActivationFunctionType
OP = mybir.AluOpType


def _ranges(n, q):
    """Split range(n) into q near-equal contiguous pieces."""
    out = []
    a = 0
    for i in range(q):
        b = a + (n - a) // (q - i)
        if b > a:
            out.append((a, b))
        a = b
    return out


def build_program(cm, n_cores):
    """cm: [BPC] uniform per-slot chunk counts (shared by all cores)."""
    cm = np.asarray(cm)
    NCHS = cm.reshape(NGRP, G)                      # [group, slot]
    NCHG = NCHS.sum(axis=1)                         # chunks per group
    NCHTOT = int(NCHG.sum())
    grp_ch0 = np.concatenate([[0], np.cumsum(NCHG)[:-1]]).astype(int)
    KMAX = max(b - a for g in range(NGRP)
               for (a, b) in _ranges(int(NCHG[g]), NRQ))

    nc = bass.Bass(num_devices=n_cores)

    # ---------------- I/O ----------------
    XTB = nc.dram_tensor("XTB", [IN_DIM, NPAD], BF16, kind="ExternalInput")
    W1AUGd = nc.dram_tensor("W1AUG", [IN_DIM, HID + 2 * H1], BF16,
                            kind="ExternalInput")
    W2AUGd = nc.dram_tensor("W2AUG", [HID, 2 + OUT_DIM], BF16,
                            kind="ExternalInput")
    B1Rd = nc.dram_tensor("B1R", [P, HID], F32, kind="ExternalInput")
    B2Rd = nc.dram_tensor("B2R", [P, OUT_DIM], F32, kind="ExternalInput")
    IOTABd = nc.dram_tensor("IOTAB", [P, KMAX * P], BF16,
                            kind="ExternalInput")
    IDENTd = nc.dram_tensor("IDENT", [P, P], BF16, kind="ExternalInput")
    ISRCd = nc.dram_tensor("ISRC", [P, NCHTOT], I32, kind="ExternalInput")
    IDSTGd = nc.dram_tensor("IDSTG", [P, NCHTOT], I32, kind="ExternalInput")
    IDSTLd = nc.dram_tensor("IDSTL", [P, NCHTOT], I32, kind="ExternalInput")
    DLOCd = nc.dram_tensor("DLOC", [P, NCHTOT], BF16, kind="ExternalInput")
    OUTd = nc.dram_tensor("OUT", [BPC * P, OUT_DIM], F32,
                          kind="ExternalOutput")

    # ---------------- internal DRAM ----------------
    T1d = nc.dram_tensor("T1", [NPAD, T1W], BF16)
    DSTTd = nc.dram_tensor("DSTT", [NPAD, DSTW], BF16)
    DSTT2d = nc.dram_tensor("DSTT2", [BPC * P, 4], F32)
    T2Ld = nc.dram_tensor("T2L", [BPC * P, T2W], F32)
    T2d = nc.dram_tensor("T2", [NPAD, T2W], F32, addr_space="Shared")

    with tile.TileContext(nc) as tc:
        with tc.tile_pool(name="consts", bufs=1) as cp:
            W1AUG_sb = cp.tile([IN_DIM, HID + 2 * H1], BF16)
            nc.sync.dma_start(out=W1AUG_sb[:], in_=W1AUGd[:, :])
            W2AUG_sb = cp.tile([HID, 2 + OUT_DIM], BF16)
            nc.sync.dma_start(out=W2AUG_sb[:], in_=W2AUGd[:, :])
            B1R_sb = cp.tile([P, HID], F32)
            nc.sync.dma_start(out=B1R_sb[:], in_=B1Rd[:, :])
            B2R_sb = cp.tile([P, OUT_DIM], F32)
            nc.sync.dma_start(out=B2R_sb[:], in_=B2Rd[:, :])
            IOTAB_sb = cp.tile([P, KMAX * P], BF16)
            nc.sync.dma_start(out=IOTAB_sb[:], in_=IOTABd[:, :])
            IDENT_sb = cp.tile([P, P], BF16)
            nc.sync.dma_start(out=IDENT_sb[:], in_=IDENTd[:, :])
            DLOC_sb = cp.tile([P, NCHTOT], BF16)
            nc.sync.dma_start(out=DLOC_sb[:], in_=DLOCd[:, :])
            ISRC_sb = cp.tile([P, NCHTOT], I32)
            nc.sync.dma_start(out=ISRC_sb[:], in_=ISRCd[:, :])
            IDSTG_sb = cp.tile([P, NCHTOT], I32)
            nc.sync.dma_start(out=IDSTG_sb[:], in_=IDSTGd[:, :])
            IDSTL_sb = cp.tile([P, NCHTOT], I32)
            nc.sync.dma_start(out=IDSTL_sb[:], in_=IDSTLd[:, :])

            # ================= Phase A: T1 / DSTT generation ================
            with tc.tile_pool(name="pa", bufs=3) as pa, \
                 tc.tile_pool(name="papsum", bufs=2, space="PSUM") as pap:
                for tb in range(NBLK // TPB):
                    c0 = tb * TPB * P
                    xt = pa.tile([IN_DIM, TPB * P], BF16, tag="xt")
                    nc.sync.dma_start(out=xt[:], in_=XTB[:, c0:c0 + TPB * P])
                    # block i at a 2KB-aligned 512-f32 stride so each matmul
                    # output sits in one PSUM bank
                    hal = pap.tile([P, TPB * 512], F32, tag="hal")
                    hal4 = hal[:].rearrange("p (t c) -> p t c", c=512)
                    for i in range(TPB):
                        nc.tensor.matmul(
                            hal[:, i * 512:i * 512 + 144],
                            lhsT=xt[:, i * P:(i + 1) * P], rhs=W1AUG_sb[:],
                            start=True, stop=True)
                    t1 = pa.tile([P, TPB, T1W], BF16, tag="t1")
                    nc.vector.tensor_copy(out=t1[:, :, 8:136],
                                          in_=hal4[:, :, 0:128])
                    nc.vector.tensor_copy(out=t1[:, :, 0:8],
                                          in_=hal4[:, :, 136:144])
                    dstt = pa.tile([P, TPB, DSTW], BF16, tag="dstt")
                    nc.vector.tensor_copy(out=dstt[:, :, 0:8],
                                          in_=hal4[:, :, 128:136])
                    nc.sync.dma_start(
                        out=T1d[c0:c0 + TPB * P, :].rearrange(
                            "(t p) e -> p t e", p=P),
                        in_=t1[:])
                    nc.scalar.dma_start(
                        out=DSTTd[c0:c0 + TPB * P, :].rearrange(
                            "(t p) e -> p t e", p=P),
                        in_=dstt[:])

            _phase_barrier(tc, nc)

            # ============ Phase B/C: edge processing (shared shape) =========
            def edge_phase(layer):
                sdt = BF16 if layer == 1 else F32
                srcw = T1W if layer == 1 else T2W
                srcwp = 144 if layer == 1 else 48   # padded tile stride
                dstw = DSTW if layer == 1 else 4
                dstwp = 16 if layer == 1 else 8     # padded tile stride
                accw = ACC1W if layer == 1 else ACC2W
                nhd = H1 if layer == 1 else 1
                fdim = HID if layer == 1 else OUT_DIM
                srcT = T1d if layer == 1 else T2d
                dstT = DSTTd if layer == 1 else DSTT2d
                ioff = IDSTG_sb if layer == 1 else IDSTL_sb
                tg = f"L{layer}"

                with tc.tile_pool(name=f"pg{layer}", bufs=3) as pg, \
                     tc.tile_pool(name=f"pm{layer}", bufs=2) as pm, \
                     tc.tile_pool(name=f"pe{layer}", bufs=2) as pe, \
                     tc.tile_pool(name=f"pp{layer}", bufs=G,
                                  space="PSUM") as pp, \
                     tc.tile_pool(name=f"pq{layer}", bufs=1,
                                  space="PSUM") as pq:
                    for g in range(NGRP):
                        ch0g = int(grp_ch0[g])
                        nchg = int(NCHG[g])
                        # chunk -> (slot-in-group, chunk-in-slot)
                        sl_of = []
                        for s7 in range(G):
                            for c in range(int(NCHS[g, s7])):
                                sl_of.append((s7, c))
                        # one PSUM bank per block slot (start=True clears the
                        # whole bank -> sibling slots must not share one)
                        accs = []
                        for _s in range(G):
                            acct = pp.tile([P, 512], F32, tag="acc",
                                           name=f"acc{layer}_{g}_{_s}")
                            accs.append(acct)
                        for (a, b) in _ranges(nchg, NRQ):
                            k = b - a
                            c0 = ch0g + a
                            # one [128,1]-offset indirect per chunk (the
                            # multi-offset form mispairs offsets on HW)
                            gt = pg.tile([P, KMAX, srcwp], sdt,
                                         tag=f"gt{tg}")
                            gd = pg.tile([P, KMAX, dstwp], sdt,
                                         tag=f"gd{tg}")
                            for j in range(k):
                                nc.gpsimd.indirect_dma_start(
                                    out=gt[:, j, 0:srcw], out_offset=None,
                                    in_=srcT[:, :],
                                    in_offset=bass.IndirectOffsetOnAxis(
                                        ap=ISRC_sb[:, c0 + j:c0 + j + 1],
                                        axis=0))
                                nc.gpsimd.indirect_dma_start(
                                    out=gd[:, j, 0:dstw], out_offset=None,
                                    in_=dstT[:, :],
                                    in_offset=bass.IndirectOffsetOnAxis(
                                        ap=ioff[:, c0 + j:c0 + j + 1],
                                        axis=0))

                            # ---- batched edge compute for this range ----
                            mask = pm.tile([P, KMAX, P], BF16,
                                           tag=f"mask{tg}")
                            nc.vector.tensor_tensor(
                                out=mask[:, 0:k, :],
                                in0=IOTAB_sb[:, 0:k * P].rearrange(
                                    "p (n d) -> p n d", d=P),
                                in1=DLOC_sb[:, c0:c0 + k]
                                    .unsqueeze(2).to_broadcast([P, k, P]),
                                op=OP.is_equal)
                            lg = pm.tile([P, KMAX, nhd], F32, tag=f"lg{tg}")
                            nc.vector.tensor_tensor(
                                out=lg[:, 0:k, :],
                                in0=gt[:, 0:k, 0:nhd],
                                in1=gd[:, 0:k, 0:nhd], op=OP.add)
                            e1 = pm.tile([P, KMAX, nhd], F32, tag=f"e1{tg}")
                            nc.scalar.activation(out=e1[:, 0:k, :],
                                                 in_=lg[:, 0:k, :],
                                                 func=AF.Exp)
                            e2 = pm.tile([P, KMAX, nhd], F32, tag=f"e2{tg}")
                            nc.scalar.activation(out=e2[:, 0:k, :],
                                                 in_=lg[:, 0:k, :],
                                                 func=AF.Exp, scale=NEG_SLOPE)
                            msg = pm.tile([P, KMAX, accw], BF16,
                                          tag=f"msg{tg}")
                            nc.vector.tensor_tensor(
                                out=msg[:, 0:k, fdim:accw],
                                in0=e1[:, 0:k, :], in1=e2[:, 0:k, :],
                                op=OP.max)
                            if layer == 1:
                                m4 = msg[:, 0:k, 0:fdim].rearrange(
                                    "p n (h c) -> p n h c", c=C1)
                                h4 = gt[:, 0:k, 8:136].rearrange(
                                    "p n (h c) -> p n h c", c=C1)
                                x4 = msg[:, 0:k, fdim:accw].unsqueeze(3) \
                                    .to_broadcast([P, k, H1, C1])
                                nc.vector.tensor_tensor(
                                    out=m4, in0=h4, in1=x4, op=OP.mult)
                            else:
                                nc.vector.tensor_tensor(
                                    out=msg[:, 0:k, 0:fdim],
                                    in0=gt[:, 0:k, 1:1 + OUT_DIM],
                                    in1=msg[:, 0:k, fdim:accw]
                                        .to_broadcast([P, k, OUT_DIM]),
                                    op=OP.mult)

                            # ---- scatter-accumulate per chunk ----
                            for j in range(a, b):
                                s7, c = sl_of[j]
                                nc.tensor.matmul(
                                    accs[s7][:, 0:accw],
                                    lhsT=mask[:, j - a, :],
                                    rhs=msg[:, j - a, :],
                                    start=(c == 0),
                                    stop=(c == int(NCHS[g, s7]) - 1),
                                    skip_group_check=True)

                        # ---- group epilogue ----
                        accsb = pe.tile([P, G, accw], F32, tag=f"accsb{tg}")
                        for s7 in range(G):
                            nc.vector.tensor_copy(
                                out=accsb[:, s7, :],
                                in_=accs[s7][:, 0:accw])
                        if layer == 1:
                            dinv = pe.tile([P, G, H1], F32, tag="dinv")
                            nc.vector.tensor_scalar(
                                out=dinv[:], in0=accsb[:, :, HID:ACC1W],
                                scalar1=EPS, scalar2=DENOM_FLOOR,
                                op0=OP.add, op1=OP.max)
                            nc.vector.reciprocal(out=dinv[:], in_=dinv[:])
                            h1 = pe.tile([P, G, HID], F32, tag="h1")
                            a4 = accsb[:, :, 0:HID].rearrange(
                                "p g (h c) -> p g h c", c=C1)
                            dv4 = dinv[:].unsqueeze(3).to_broadcast(
                                [P, G, H1, C1])
                            h14 = h1[:].rearrange(
                                "p g (h c) -> p g h c", c=C1)
                            nc.vector.tensor_tensor(
                                out=h14, in0=a4, in1=dv4, op=OP.mult)
                            nc.vector.tensor_tensor(
                                out=h1[:], in0=h1[:],
                                in1=B1R_sb[:].unsqueeze(1)
                                    .to_broadcast([P, G, HID]),
                                op=OP.add)
                            # ELU(x) = max(x,0) + min(exp(x)-1, 0)
                            ex = pe.tile([P, G, HID], F32, tag="ex")
                            nc.scalar.activation(out=ex[:], in_=h1[:],
                                                 func=AF.Exp)
                            nc.vector.tensor_scalar(
                                out=ex[:], in0=ex[:], scalar1=-1.0,
                                scalar2=0.0, op0=OP.add, op1=OP.min)
                            nc.vector.tensor_scalar(
                                out=h1[:], in0=h1[:], scalar1=0.0,
                                scalar2=None, op0=OP.max)
                            h1e = pe.tile([P, G, HID], BF16, tag="h1e")
                            nc.vector.tensor_tensor(out=h1e[:], in0=h1[:],
                                                    in1=ex[:], op=OP.add)
                            for s7 in range(G):
                                s = g * G + s7
                                epi = pq.tile([P, 1024], BF16, tag="epi",
                                              name=f"epi_{g}_{s7}")
                                h1T_ps = epi[:, 0:HID]
                                nc.tensor.transpose(h1T_ps,
                                                    h1e[:, s7, :],
                                                    IDENT_sb[:])
                                h1T = pe.tile([P, HID], BF16, tag="h1Ts")
                                nc.vector.tensor_copy(out=h1T[:],
                                                      in_=h1T_ps)
                                hg2x = epi[:, 256:256 + 2 * (2 + OUT_DIM)]
                                hg2 = hg2x.bitcast(F32)
                                nc.tensor.matmul(hg2, lhsT=h1T[:],
                                                 rhs=W2AUG_sb[:],
                                                 start=True, stop=True)
                                t2row = pe.tile([P, T2W], F32, tag="t2r")
                                nc.vector.tensor_copy(
                                    out=t2row[:],
                                    in_=hg2[:, 1:2 + OUT_DIM])
                                d2row = pe.tile([P, 4], F32, tag="d2r")
                                nc.vector.tensor_copy(out=d2row[:, 0:1],
                                                      in_=hg2[:, 0:1])
                                nc.scalar.dma_start(
                                    out=T2Ld[s * P:(s + 1) * P, :],
                                    in_=t2row[:])
                                nc.scalar.dma_start(
                                    out=DSTT2d[s * P:(s + 1) * P, :],
                                    in_=d2row[:])
                        else:
                            dinv2 = pe.tile([P, G, 1], F32, tag="dinv2")
                            nc.vector.tensor_scalar(
                                out=dinv2[:], in0=accsb[:, :, OUT_DIM:ACC2W],
                                scalar1=EPS, scalar2=DENOM_FLOOR,
                                op0=OP.add, op1=OP.max)
                            nc.vector.reciprocal(out=dinv2[:], in_=dinv2[:])
                            o = pe.tile([P, G, OUT_DIM], F32, tag="o")
                            nc.vector.tensor_tensor(
                                out=o[:], in0=accsb[:, :, 0:OUT_DIM],
                                in1=dinv2[:].to_broadcast([P, G, OUT_DIM]),
                                op=OP.mult)
                            nc.vector.tensor_tensor(
                                out=o[:], in0=o[:],
                                in1=B2R_sb[:].unsqueeze(1)
                                    .to_broadcast([P, G, OUT_DIM]),
                                op=OP.add)
                            # log_softmax = (o - m) - ln(sum(exp(o - m)))
                            nm = pe.tile([P, G, 1], F32, tag="nm")
                            nc.vector.tensor_reduce(
                                out=nm[:], in_=o[:],
                                axis=mybir.AxisListType.X,
                                op=OP.max, negate=True)
                            osh = pe.tile([P, G, OUT_DIM], F32, tag="osh")
                            nc.vector.tensor_tensor(
                                out=osh[:], in0=o[:],
                                in1=nm[:].to_broadcast([P, G, OUT_DIM]),
                                op=OP.add)
                            e2t = pe.tile([P, G, OUT_DIM], F32, tag="e2t")
                            nc.scalar.activation(out=e2t[:], in_=osh[:],
                                                 func=AF.Exp)
                            s2 = pe.tile([P, G, 1], F32, tag="s2")
                            nc.vector.tensor_reduce(
                                out=s2[:], in_=e2t[:],
                                axis=mybir.AxisListType.X, op=OP.add)
                            ls = pe.tile([P, G, 1], F32, tag="ls")
                            nc.scalar.activation(out=ls[:], in_=s2[:],
                                                 func=AF.Ln)
                            ot = pe.tile([P, G, OUT_DIM], F32, tag="ot")
                            nc.vector.tensor_tensor(
                                out=ot[:], in0=osh[:],
                                in1=ls[:].to_broadcast([P, G, OUT_DIM]),
                                op=OP.subtract)
                            nc.sync.dma_start(
                                out=OUTd[g * GROWS:(g + 1) * GROWS, :]
                                    .rearrange("(t p) e -> p t e", p=P),
                                in_=ot[:])

            edge_phase(1)

            # ============= AllGather of T2 shards ===============
            nc.gpsimd.collective_compute(
                "AllGather", OP.bypass,
                replica_groups=[list(range(n_cores))],
                ins=[T2Ld[:, :].opt()],
                outs=[T2d[:, :].opt()],
            )
            _phase_barrier(tc, nc)

            edge_phase(2)

    return nc


# ----------------------------------------------------------------------------
# host-side preprocessing (index/layout work)
# ----------------------------------------------------------------------------

def preprocess_graph(src, dst, n_nodes):
    """Assign nodes to in-degree-balanced blocks of 128."""
    deg = np.bincount(dst, minlength=n_nodes)
    order = np.argsort(-deg, kind="stable")
    r = np.arange(n_nodes)
    rounds, posr = r // NBLK, r % NBLK
    binr = np.where(rounds % 2 == 0, posr, NBLK - 1 - posr)
    blk_of_node = np.empty(n_nodes, np.int64)
    blk_of_node[order] = binr
    cnt = np.bincount(blk_of_node, minlength=NBLK)
    assert cnt.max() <= P, f"block overfull: {cnt.max()}"
    node_sorted = np.argsort(blk_of_node, kind="stable")
    starts = np.concatenate([[0], np.cumsum(cnt)[:-1]])
    slot_sorted = np.arange(n_nodes) - np.repeat(starts, cnt)
    slot_of_node = np.empty(n_nodes, np.int64)
    slot_of_node[node_sorted] = slot_sorted
    pos_of_node = blk_of_node * P + slot_of_node
    return pos_of_node, blk_of_node, slot_of_node


def build_edge_tables(src, dst, pos, blk, slot):
    """Uniform chunk grid + per-core offset/dloc tables."""
    dblk = blk[dst]
    core = dblk // BPC
    dslot = dblk % BPC
    spos = pos[src]

    key = core * BPC + dslot
    order = np.argsort(key, kind="stable")
    cnts = np.bincount(key, minlength=N_CORES * BPC)
    ch = np.ceil(cnts / P).astype(np.int64).reshape(N_CORES, BPC)
    cm = np.maximum(ch.max(axis=0), 1)              # [BPC]

    NCHTOT = int(cm.sum())
    starts = np.concatenate([[0], np.cumsum(cnts)[:-1]])
    chcol_of_slot = np.concatenate([[0], np.cumsum(cm)[:-1]])

    dst_sl = slot[dst].astype(np.int32)

    per_core = []
    for k in range(N_CORES):
        isrc = np.zeros((P, NCHTOT), np.int32)
        idstg = np.zeros((P, NCHTOT), np.int32)
        idstl = np.zeros((P, NCHTOT), np.int32)
        dloc = np.full((P, NCHTOT), PADLOC, np.float32)
        for s in range(BPC):
            ki = k * BPC + s
            n = cnts[ki]
            e = order[starts[ki]:starts[ki] + n]
            chcol = chcol_of_slot[s]
            cols = chcol + np.arange(n) // P
            rows = np.arange(n) % P
            isrc[rows, cols] = spos[e]
            idstg[rows, cols] = (k * BPC + s) * P + dst_sl[e]
            idstl[rows, cols] = s * P + dst_sl[e]
            dloc[rows, cols] = dst_sl[e]
        per_core.append({
            "ISRC": isrc,
            "IDSTG": idstg,
            "IDSTL": idstl,
            "DLOC": dloc.astype(ml_dtypes.bfloat16),
        })
    return cm, per_core


def build_inputs(x, edge_index, W1, a_src1, a_dst1, b1, W2, a_src2, a_dst2,
                 b2, n_cores):
    src = np.asarray(edge_index[0], dtype=np.int64)
    dst = np.asarray(edge_index[1], dtype=np.int64)
    pos, blk, slot = preprocess_graph(src, dst, N_NODES)
    cm, per_core = build_edge_tables(src, dst, pos, blk, slot)
    NCHG = cm.reshape(NGRP, G).sum(axis=1)
    KMAX = max(b - a for g in range(NGRP)
               for (a, b) in _ranges(int(NCHG[g]), NRQ))

    x = np.asarray(x, np.float32)
    XTa = np.zeros((IN_DIM, NPAD), np.float32)
    XTa[:, pos] = x.T

    W1 = np.asarray(W1, np.float32)
    W2 = np.asarray(W2, np.float32)
    a_src1 = np.asarray(a_src1, np.float32)
    a_dst1 = np.asarray(a_dst1, np.float32)
    a_src2 = np.asarray(a_src2, np.float32)
    a_dst2 = np.asarray(a_dst2, np.float32)
    b1 = np.asarray(b1, np.float32)
    b2 = np.asarray(b2, np.float32)

    # A1BD columns: [adst1(8) | asrc1(8)] per-head block-diagonal
    A1BD = np.zeros((HID, 2 * H1), np.float32)
    for h in range(H1):
        A1BD[h * C1:(h + 1) * C1, h] = a_dst1[h]
        A1BD[h * C1:(h + 1) * C1, H1 + h] = a_src1[h]
    W1AUG = np.concatenate([W1, W1 @ A1BD], axis=1)

    A2T = np.stack([a_dst2[0], a_src2[0]], axis=1)     # [OUT_DIM, 2]
    W2AUG = np.concatenate([W2 @ A2T, W2], axis=1)     # [HID, 2+OUT_DIM]

    iota = np.broadcast_to(np.arange(P, dtype=np.float32), (P, P))
    iotab = np.tile(iota, (1, KMAX))

    common = {
        "XTB": XTa.astype(ml_dtypes.bfloat16),
        "W1AUG": W1AUG.astype(ml_dtypes.bfloat16),
        "W2AUG": W2AUG.astype(ml_dtypes.bfloat16),
        "B1R": np.ascontiguousarray(np.broadcast_to(b1, (P, HID))),
        "B2R": np.ascontiguousarray(np.broadcast_to(b2, (P, OUT_DIM))),
        "IOTAB": np.ascontiguousarray(iotab).astype(ml_dtypes.bfloat16),
        "IDENT": np.eye(P, dtype=ml_dtypes.bfloat16),
    }
    in_maps = [dict(common, **pc) for pc in per_core]
    return in_maps, pos, cm


# ----------------------------------------------------------------------------
# entry point
# ----------------------------------------------------------------------------

_prog_cache = {}
last_results = None


def _get_program(cm, n_cores):
    key = (cm.tobytes(), n_cores)
    if key not in _prog_cache:
        nc = build_program(cm, n_cores)
        _split_excess_waits(nc)
        _prog_cache[key] = nc
    return _prog_cache[key]


def run(inputs, n_cores=N_CORES, trace=False):
    global last_results
    in_maps, pos, cm = build_inputs(n_cores=n_cores, **inputs)
    nc = _get_program(cm, n_cores)
    kwargs = {}
    if trace:
        kwargs = dict(trace=True, trace_cores=[0])
    res = run_bass_kernel_spmd(
        nc, in_maps, core_ids=list(range(n_cores)), **kwargs)
    last_results = res
    out_all = np.concatenate([r["OUT"] for r in res.results], axis=0)
    return np.ascontiguousarray(out_all[pos].astype(np.float32))


def kernel(**inputs):
    return run(inputs)


# revision 15
# speedup vs baseline: 1.0778x; 1.0778x over previous
"""Trainium2 Bass kernel: 2-layer GAT (nn_GAT_1709396983866), v3.

Strategy (graph/data parallel over 8 NeuronCores):
  * Nodes are permuted and packed into 784 blocks of 128 positions, balanced
    by in-degree (~2041 edges/block). Core k owns blocks [98k, 98k+98); edges
    are sharded by destination block so segment-softmax / scatter-add stay
    core-local. Each block's edges form chunks of 128 (same dst block); the
    per-slot chunk counts are padded to a cross-core-uniform grid `cm` so all
    8 cores run one SPMD program.
  * Per-edge feature gathers use multi-offset `indirect_dma_start`: one call
    gathers ~28 chunks' worth of rows (int32 offsets, [128, k] offset AP)
    instead of v1's one call per chunk — the ~1us SWDGE fixed cost (the v1
    bottleneck: 6272 calls x 1.09us serialized on GpSimd) is amortized ~28x.
  * Gather tables:
      T1    [npad, 136] bf16: [asrc1(8) | h(128)]      (by src, layer 1)
      DSTT  [npad, 8]   bf16: adst1                    (by dst, layer 1)
      T2    [npad, 41]  f32:  [asrc2(1) | h2(40)]      (by src, layer 2)
      DSTT2 [12544, 1]  f32:  adst2       (core-local, by dst, layer 2)
    T1/DSTT are computed replicated in phase A; T2 shards are AllGathered.
  * Per chunk: one-hot mask[e,d] = (dst_local[e]==d); logits = asrc[src] +
    adst[dst] from the two gathers; exp(leaky_relu(x)) = max(exp(x),
    exp(0.2x)) (exact, by monotonicity); messages [h*exp | exp] are
    scatter-added per dst block via mask.T @ msg on the tensor engine (PSUM
    accumulation, one bank per block slot so start=True bank-clears can't
    clobber a sibling accumulation).
  * Blocks are processed in groups of 7, chunks in ranges of ~28; all
    elementwise work is batched per range (~3600 elems/partition per DVE
    instruction) to amortize the ~150-290ns per-instruction overheads.

kernel(**inputs) takes the full unsharded inputs and returns the full output.
"""

import numpy as np
import ml_dtypes

import concourse.bass as bass
import concourse.tile as tile
from concourse import mybir
from concourse.bass_utils import run_bass_kernel_spmd
from concourse.tile_rust import add_dep_helper


# Per-opcode embedded sync-wait slot budget in walrus codegen (empirical).
_WAIT_LIMITS = {}
_WAIT_LIMIT_DEFAULT = 1
_NOSPLIT_OPS = ("EventSemaphore",)


def _split_excess_waits(nc):
    """Move excess sem waits onto preceding same-engine wait instructions."""
    nid = [0]

    def mk_wait(engine, wait):
        nid[0] += 1
        ev = mybir.InstEventSemaphore(
            name=f"waitsplit-{nid[0]}", ins=[], outs=[])
        ev.engine = engine
        ev.sync_info = mybir.SyncInfo(on_wait=[wait], on_update=[])
        return ev

    for fn in nc.m.functions:
        for bb in fn.blocks:
            out = []
            for inst in bb.instructions:
                si = inst.sync_info
                waits = list(si.on_wait) if si and si.on_wait else []
                lim = _WAIT_LIMITS.get(inst.opcode, _WAIT_LIMIT_DEFAULT)
                if len(waits) > lim and inst.opcode not in _NOSPLIT_OPS:
                    excess, keep = waits[:-lim], waits[-lim:]
                    for w in excess:
                        out.append(mk_wait(inst.engine, w))
                    inst.sync_info = mybir.SyncInfo(
                        on_wait=keep, on_update=list(si.on_update or []))
                out.append(inst)
            bb.instructions = out


def _phase_barrier(tc, nc):
    """All-engine barrier that soaks per-DMA-lane waits per engine first."""
    curr_bb = nc.cur_bb
    prev = list(curr_bb.bb.instructions)
    for eng in (nc.gpsimd, nc.sync, nc.scalar, nc.vector, nc.tensor):
        nop = eng.nop()
        for inst in prev:
            add_dep_helper(
                nop.ins, inst,
                sync=bass.sync_unless_reorderable_target(
                    inst, inst.is_executable()),
                reason="phase-barrier soak")
    tc.strict_bb_all_engine_barrier()


# -------- problem constants (hardcoded, per spec) --------
N_NODES = 100000
IN_DIM = 128
HID = 128
OUT_DIM = 40
H1 = 8
C1 = 16
NEG_SLOPE = 0.2
EPS = 1e-16
DENOM_FLOOR = 1e-6
N_CORES = 8
P = 128
NBLK = 784
BPC = NBLK // N_CORES          # 98
NPAD = NBLK * P                # 100352
G = 7                          # block slots per group
NGRP = BPC // G                # 14
GROWS = G * P                  # 896 rows per group
PADLOC = 200.0                 # dst_local for padding edge slots
TPB = 2                        # blocks per phase-A batch
NRQ = 4                        # chunk ranges per group (gather call batches)

T1W = H1 + HID                 # 136: [asrc1 | h]
DSTW = H1                      # 8: adst1
T2W = 1 + OUT_DIM              # 41: [asrc2 | h2]
ACC1W = HID + H1               # 136
ACC2W = OUT_DIM + 1            # 41

F32 = mybir.dt.float32
BF16 = mybir.dt.bfloat16
I32 = mybir.dt.int32
AF = mybir.ActivationFunctionType
OP = mybir.AluOpType


def _ranges(n, q):
    """Split range(n) into q near-equal contiguous pieces."""
    out = []
    a = 0
    for i in range(q):
        b = a + (n - a) // (q - i)
        if b > a:
            out.append((a, b))
        a = b
    return out


def build_program(cm, n_cores):
    """cm: [BPC] uniform per-slot chunk counts (shared by all cores)."""
    cm = np.asarray(cm)
    NCHS = cm.reshape(NGRP, G)                      # [group, slot]
    NCHG = NCHS.sum(axis=1)                         # chunks per group
    NCHTOT = int(NCHG.sum())
    grp_ch0 = np.concatenate([[0], np.cumsum(NCHG)[:-1]]).astype(int)
    KMAX = max(b - a for g in range(NGRP)
               for (a, b) in _ranges(int(NCHG[g]), NRQ))

    nc = bass.Bass(num_devices=n_cores)

    # ---------------- I/O ----------------
    XTB = nc.dram_tensor("XTB", [IN_DIM, BPC * P], BF16,
                          kind="ExternalInput")
    W1AUGd = nc.dram_tensor("W1AUG", [IN_DIM, HID + 2 * H1], BF16,
                            kind="ExternalInput")
    W2AUGd = nc.dram_tensor("W2AUG", [HID, 2 + OUT_DIM], BF16,
                            kind="ExternalInput")
    B1Rd = nc.dram_tensor("B1R", [P, HID], F32, kind="ExternalInput")
    B2Rd = nc.dram_tensor("B2R", [P, OUT_DIM], F32, kind="ExternalInput")
    IOTABd = nc.dram_tensor("IOTAB", [P, KMAX * P], BF16,
                            kind="ExternalInput")
    IDENTd = nc.dram_tensor("IDENT", [P, P], BF16, kind="ExternalInput")
    ISRCd = nc.dram_tensor("ISRC", [P, NCHTOT], I32, kind="ExternalInput")
    IDSTGd = nc.dram_tensor("IDSTG", [P, NCHTOT], I32, kind="ExternalInput")
    IDSTLd = nc.dram_tensor("IDSTL", [P, NCHTOT], I32, kind="ExternalInput")
    DLOCd = nc.dram_tensor("DLOC", [P, NCHTOT], BF16, kind="ExternalInput")
    OUTd = nc.dram_tensor("OUT", [BPC * P, OUT_DIM], F32,
                          kind="ExternalOutput")

    # ---------------- internal DRAM ----------------
    T1Ld = nc.dram_tensor("T1L", [BPC * P, T1W], BF16)
    DSTTLd = nc.dram_tensor("DSTTL", [BPC * P, DSTW], BF16)
    T1d = nc.dram_tensor("T1", [NPAD, T1W], BF16, addr_space="Shared")
    DSTTd = nc.dram_tensor("DSTT", [NPAD, DSTW], BF16, addr_space="Shared")
    DSTT2d = nc.dram_tensor("DSTT2", [BPC * P, 4], F32)
    T2Ld = nc.dram_tensor("T2L", [BPC * P, T2W], F32)
    T2d = nc.dram_tensor("T2", [NPAD, T2W], F32, addr_space="Shared")

    with tile.TileContext(nc) as tc:
        with tc.tile_pool(name="consts", bufs=1) as cp:
            W1AUG_sb = cp.tile([IN_DIM, HID + 2 * H1], BF16)
            nc.sync.dma_start(out=W1AUG_sb[:], in_=W1AUGd[:, :])
            W2AUG_sb = cp.tile([HID, 2 + OUT_DIM], BF16)
            nc.sync.dma_start(out=W2AUG_sb[:], in_=W2AUGd[:, :])
            B1R_sb = cp.tile([P, HID], F32)
            nc.sync.dma_start(out=B1R_sb[:], in_=B1Rd[:, :])
            B2R_sb = cp.tile([P, OUT_DIM], F32)
            nc.sync.dma_start(out=B2R_sb[:], in_=B2Rd[:, :])
            IOTAB_sb = cp.tile([P, KMAX * P], BF16)
            nc.sync.dma_start(out=IOTAB_sb[:], in_=IOTABd[:, :])
            IDENT_sb = cp.tile([P, P], BF16)
            nc.sync.dma_start(out=IDENT_sb[:], in_=IDENTd[:, :])
            DLOC_sb = cp.tile([P, NCHTOT], BF16)
            nc.sync.dma_start(out=DLOC_sb[:], in_=DLOCd[:, :])
            ISRC_sb = cp.tile([P, NCHTOT], I32)
            nc.sync.dma_start(out=ISRC_sb[:], in_=ISRCd[:, :])
            IDSTG_sb = cp.tile([P, NCHTOT], I32)
            nc.sync.dma_start(out=IDSTG_sb[:], in_=IDSTGd[:, :])
            IDSTL_sb = cp.tile([P, NCHTOT], I32)
            nc.sync.dma_start(out=IDSTL_sb[:], in_=IDSTLd[:, :])

            # ================= Phase A: T1 / DSTT generation ================
            with tc.tile_pool(name="pa", bufs=3) as pa, \
                 tc.tile_pool(name="papsum", bufs=2, space="PSUM") as pap:
                for tb in range(BPC // TPB):
                    c0 = tb * TPB * P
                    xt = pa.tile([IN_DIM, TPB * P], BF16, tag="xt")
                    nc.sync.dma_start(out=xt[:], in_=XTB[:, c0:c0 + TPB * P])
                    # block i at a 2KB-aligned 512-f32 stride so each matmul
                    # output sits in one PSUM bank
                    hal = pap.tile([P, TPB * 512], F32, tag="hal")
                    hal4 = hal[:].rearrange("p (t c) -> p t c", c=512)
                    for i in range(TPB):
                        nc.tensor.matmul(
                            hal[:, i * 512:i * 512 + 144],
                            lhsT=xt[:, i * P:(i + 1) * P], rhs=W1AUG_sb[:],
                            start=True, stop=True)
                    t1 = pa.tile([P, TPB, T1W], BF16, tag="t1")
                    nc.vector.tensor_copy(out=t1[:, :, 8:136],
                                          in_=hal4[:, :, 0:128])
                    nc.vector.tensor_copy(out=t1[:, :, 0:8],
                                          in_=hal4[:, :, 136:144])
                    dstt = pa.tile([P, TPB, DSTW], BF16, tag="dstt")
                    nc.vector.tensor_copy(out=dstt[:, :, 0:8],
                                          in_=hal4[:, :, 128:136])
                    nc.sync.dma_start(
                        out=T1Ld[c0:c0 + TPB * P, :].rearrange(
                            "(t p) e -> p t e", p=P),
                        in_=t1[:])
                    nc.scalar.dma_start(
                        out=DSTTLd[c0:c0 + TPB * P, :].rearrange(
                            "(t p) e -> p t e", p=P),
                        in_=dstt[:])

            # AllGather the phase-A table shards
            nc.gpsimd.collective_compute(
                "AllGather", OP.bypass,
                replica_groups=[list(range(n_cores))],
                ins=[T1Ld[:, :].opt()],
                outs=[T1d[:, :].opt()],
            )
            nc.gpsimd.collective_compute(
                "AllGather", OP.bypass,
                replica_groups=[list(range(n_cores))],
                ins=[DSTTLd[:, :].opt()],
                outs=[DSTTd[:, :].opt()],
            )

            _phase_barrier(tc, nc)

            # ============ Phase B/C: edge processing (shared shape) =========
            def edge_phase(layer):
                sdt = BF16 if layer == 1 else F32
                srcw = T1W if layer == 1 else T2W
                srcwp = 144 if layer == 1 else 48   # padded tile stride
                dstw = DSTW if layer == 1 else 4
                dstwp = 16 if layer == 1 else 8     # padded tile stride
                accw = ACC1W if layer == 1 else ACC2W
                nhd = H1 if layer == 1 else 1
                fdim = HID if layer == 1 else OUT_DIM
                srcT = T1d if layer == 1 else T2d
                dstT = DSTTd if layer == 1 else DSTT2d
                ioff = IDSTG_sb if layer == 1 else IDSTL_sb
                tg = f"L{layer}"

                with tc.tile_pool(name=f"pg{layer}", bufs=3) as pg, \
                     tc.tile_pool(name=f"pm{layer}", bufs=2) as pm, \
                     tc.tile_pool(name=f"pe{layer}", bufs=2) as pe, \
                     tc.tile_pool(name=f"pp{layer}", bufs=G,
                                  space="PSUM") as pp, \
                     tc.tile_pool(name=f"pq{layer}", bufs=1,
                                  space="PSUM") as pq:
                    for g in range(NGRP):
                        ch0g = int(grp_ch0[g])
                        nchg = int(NCHG[g])
                        # chunk -> (slot-in-group, chunk-in-slot)
                        sl_of = []
                        for s7 in range(G):
                            for c in range(int(NCHS[g, s7])):
                                sl_of.append((s7, c))
                        # one PSUM bank per block slot (start=True clears the
                        # whole bank -> sibling slots must not share one)
                        accs = []
                        for _s in range(G):
                            acct = pp.tile([P, 512], F32, tag="acc",
                                           name=f"acc{layer}_{g}_{_s}")
                            accs.append(acct)
                        for (a, b) in _ranges(nchg, NRQ):
                            k = b - a
                            c0 = ch0g + a
                            # one [128,1]-offset indirect per chunk (the
                            # multi-offset form mispairs offsets on HW)
                            gt = pg.tile([P, KMAX, srcwp], sdt,
                                         tag=f"gt{tg}")
                            gd = pg.tile([P, KMAX, dstwp], sdt,
                                         tag=f"gd{tg}")
                            for j in range(k):
                                nc.gpsimd.indirect_dma_start(
                                    out=gt[:, j, 0:srcw], out_offset=None,
                                    in_=srcT[:, :],
                                    in_offset=bass.IndirectOffsetOnAxis(
                                        ap=ISRC_sb[:, c0 + j:c0 + j + 1],
                                        axis=0))
                                nc.gpsimd.indirect_dma_start(
                                    out=gd[:, j, 0:dstw], out_offset=None,
                                    in_=dstT[:, :],
                                    in_offset=bass.IndirectOffsetOnAxis(
                                        ap=ioff[:, c0 + j:c0 + j + 1],
                                        axis=0))

                            # ---- batched edge compute for this range ----
                            mask = pm.tile([P, KMAX, P], BF16,
                                           tag=f"mask{tg}")
                            nc.vector.tensor_tensor(
                                out=mask[:, 0:k, :],
                                in0=IOTAB_sb[:, 0:k * P].rearrange(
                                    "p (n d) -> p n d", d=P),
                                in1=DLOC_sb[:, c0:c0 + k]
                                    .unsqueeze(2).to_broadcast([P, k, P]),
                                op=OP.is_equal)
                            lg = pm.tile([P, KMAX, nhd], F32, tag=f"lg{tg}")
                            nc.vector.tensor_tensor(
                                out=lg[:, 0:k, :],
                                in0=gt[:, 0:k, 0:nhd],
                                in1=gd[:, 0:k, 0:nhd], op=OP.add)
                            e1 = pm.tile([P, KMAX, nhd], F32, tag=f"e1{tg}")
                            nc.scalar.activation(out=e1[:, 0:k, :],
                                                 in_=lg[:, 0:k, :],
                                                 func=AF.Exp)
                            e2 = pm.tile([P, KMAX, nhd], F32, tag=f"e2{tg}")
                            nc.scalar.activation(out=e2[:, 0:k, :],
                                                 in_=lg[:, 0:k, :],
                                                 func=AF.Exp, scale=NEG_SLOPE)
                            msg = pm.tile([P, KMAX, accw], BF16,
                                          tag=f"msg{tg}")
                            nc.vector.tensor_tensor(
                                out=msg[:, 0:k, fdim:accw],
                                in0=e1[:, 0:k, :], in1=e2[:, 0:k, :],
                                op=OP.max)
                            if layer == 1:
                                m4 = msg[:, 0:k, 0:fdim].rearrange(
                                    "p n (h c) -> p n h c", c=C1)
                                h4 = gt[:, 0:k, 8:136].rearrange(
                                    "p n (h c) -> p n h c", c=C1)
                                x4 = msg[:, 0:k, fdim:accw].unsqueeze(3) \
                                    .to_broadcast([P, k, H1, C1])
                                nc.vector.tensor_tensor(
                                    out=m4, in0=h4, in1=x4, op=OP.mult)
                            else:
                                nc.vector.tensor_tensor(
                                    out=msg[:, 0:k, 0:fdim],
                                    in0=gt[:, 0:k, 1:1 + OUT_DIM],
                                    in1=msg[:, 0:k, fdim:accw]
                                        .to_broadcast([P, k, OUT_DIM]),
                                    op=OP.mult)

                            # ---- scatter-accumulate per chunk ----
                            for j in range(a, b):
                                s7, c = sl_of[j]
                                nc.tensor.matmul(
                                    accs[s7][:, 0:accw],
                                    lhsT=mask[:, j - a, :],
                                    rhs=msg[:, j - a, :],
                                    start=(c == 0),
                                    stop=(c == int(NCHS[g, s7]) - 1),
                                    skip_group_check=True)

                        # ---- group epilogue ----
                        accsb = pe.tile([P, G, accw], F32, tag=f"accsb{tg}")
                        for s7 in range(G):
                            nc.vector.tensor_copy(
                                out=accsb[:, s7, :],
                                in_=accs[s7][:, 0:accw])
                        if layer == 1:
                            dinv = pe.tile([P, G, H1], F32, tag="dinv")
                            nc.vector.tensor_scalar(
                                out=dinv[:], in0=accsb[:, :, HID:ACC1W],
                                scalar1=EPS, scalar2=DENOM_FLOOR,
                                op0=OP.add, op1=OP.max)
                            nc.vector.reciprocal(out=dinv[:], in_=dinv[:])
                            h1 = pe.tile([P, G, HID], F32, tag="h1")
                            a4 = accsb[:, :, 0:HID].rearrange(
                                "p g (h c) -> p g h c", c=C1)
                            dv4 = dinv[:].unsqueeze(3).to_broadcast(
                                [P, G, H1, C1])
                            h14 = h1[:].rearrange(
                                "p g (h c) -> p g h c", c=C1)
                            nc.vector.tensor_tensor(
                                out=h14, in0=a4, in1=dv4, op=OP.mult)
                            nc.vector.tensor_tensor(
                                out=h1[:], in0=h1[:],
                                in1=B1R_sb[:].unsqueeze(1)
                                    .to_broadcast([P, G, HID]),
                                op=OP.add)
                            # ELU(x) = max(x,0) + min(exp(x)-1, 0)
                            ex = pe.tile([P, G, HID], F32, tag="ex")
                            nc.scalar.activation(out=ex[:], in_=h1[:],
                                                 func=AF.Exp)
                            nc.vector.tensor_scalar(
                                out=ex[:], in0=ex[:], scalar1=-1.0,
                                scalar2=0.0, op0=OP.add, op1=OP.min)
                            nc.vector.tensor_scalar(
                                out=h1[:], in0=h1[:], scalar1=0.0,
                                scalar2=None, op0=OP.max)
                            h1e = pe.tile([P, G, HID], BF16, tag="h1e")
                            nc.vector.tensor_tensor(out=h1e[:], in0=h1[:],
                                                    in1=ex[:], op=OP.add)
                            for s7 in range(G):
                                s = g * G + s7
                                epi = pq.tile([P, 1024], BF16, tag="epi",
                                              name=f"epi_{g}_{s7}")
                                h1T_ps = epi[:, 0:HID]
                                nc.tensor.transpose(h1T_ps,
                                                    h1e[:, s7, :],
                                                    IDENT_sb[:])
                                h1T = pe.tile([P, HID], BF16, tag="h1Ts")
                                nc.vector.tensor_copy(out=h1T[:],
                                                      in_=h1T_ps)
                                hg2x = epi[:, 256:256 + 2 * (2 + OUT_DIM)]
                                hg2 = hg2x.bitcast(F32)
                                nc.tensor.matmul(hg2, lhsT=h1T[:],
                                                 rhs=W2AUG_sb[:],
                                                 start=True, stop=True)
                                t2row = pe.tile([P, T2W], F32, tag="t2r")
                                nc.vector.tensor_copy(
                                    out=t2row[:],
                                    in_=hg2[:, 1:2 + OUT_DIM])
                                d2row = pe.tile([P, 4], F32, tag="d2r")
                                nc.vector.tensor_copy(out=d2row[:, 0:1],
                                                      in_=hg2[:, 0:1])
                                nc.scalar.dma_start(
                                    out=T2Ld[s * P:(s + 1) * P, :],
                                    in_=t2row[:])
                                nc.scalar.dma_start(
                                    out=DSTT2d[s * P:(s + 1) * P, :],
                                    in_=d2row[:])
                        else:
                            dinv2 = pe.tile([P, G, 1], F32, tag="dinv2")
                            nc.vector.tensor_scalar(
                                out=dinv2[:], in0=accsb[:, :, OUT_DIM:ACC2W],
                                scalar1=EPS, scalar2=DENOM_FLOOR,
                                op0=OP.add, op1=OP.max)
                            nc.vector.reciprocal(out=dinv2[:], in_=dinv2[:])
                            o = pe.tile([P, G, OUT_DIM], F32, tag="o")
                            nc.vector.tensor_tensor(
                                out=o[:], in0=accsb[:, :, 0:OUT_DIM],
                                in1=dinv2[:].to_broadcast([P, G, OUT_DIM]),
                                op=OP.mult)
                            nc.vector.tensor_tensor(
                                out=o[:], in0=o[:],
                                in1=B2R_sb[:].unsqueeze(1)
                                    .to_broadcast([P, G, OUT_DIM]),
                                op=OP.add)
                            # log_softmax = (o - m) - ln(sum(exp(o - m)))
                            nm = pe.tile([P, G, 1], F32, tag="nm")
                            nc.vector.tensor_reduce(
                                out=nm[:], in_=o[:],
                                axis=mybir.AxisListType.X,
                                op=OP.max, negate=True)
                            osh = pe.tile([P, G, OUT_DIM], F32, tag="osh")
                            nc.vector.tensor_tensor(
                                out=osh[:], in0=o[:],
                                in1=nm[:].to_broadcast([P, G, OUT_DIM]),
                                op=OP.add)
                            e2t = pe.tile([P, G, OUT_DIM], F32, tag="e2t")
                            nc.scalar.activation(out=e2t[:], in_=osh[:],
                                                 func=AF.Exp)
                            s2 = pe.tile([P, G, 1], F32, tag="s2")
                            nc.vector.tensor_reduce(
                                out=s2[:], in_=e2t[:],
                                axis=mybir.AxisListType.X, op=OP.add)
                            ls = pe.tile([P, G, 1], F32, tag="ls")
                            nc.scalar.activation(out=ls[:], in_=s2[:],
                                                 func=AF.Ln)
                            ot = pe.tile([P, G, OUT_DIM], F32, tag="ot")
                            nc.vector.tensor_tensor(
                                out=ot[:], in0=osh[:],
                                in1=ls[:].to_broadcast([P, G, OUT_DIM]),
                                op=OP.subtract)
                            nc.sync.dma_start(
                                out=OUTd[g * GROWS:(g + 1) * GROWS, :]
                                    .rearrange("(t p) e -> p t e", p=P),
                                in_=ot[:])

            edge_phase(1)

            # ============= AllGather of T2 shards ===============
            nc.gpsimd.collective_compute(
                "AllGather", OP.bypass,
                replica_groups=[list(range(n_cores))],
                ins=[T2Ld[:, :].opt()],
                outs=[T2d[:, :].opt()],
            )
            _phase_barrier(tc, nc)

            edge_phase(2)

    return nc


# ----------------------------------------------------------------------------
# host-side preprocessing (index/layout work)
# ----------------------------------------------------------------------------

def preprocess_graph(src, dst, n_nodes):
    """Assign nodes to in-degree-balanced blocks of 128."""
    deg = np.bincount(dst, minlength=n_nodes)
    order = np.argsort(-deg, kind="stable")
    r = np.arange(n_nodes)
    rounds, posr = r // NBLK, r % NBLK
    binr = np.where(rounds % 2 == 0, posr, NBLK - 1 - posr)
    blk_of_node = np.empty(n_nodes, np.int64)
    blk_of_node[order] = binr
    cnt = np.bincount(blk_of_node, minlength=NBLK)
    assert cnt.max() <= P, f"block overfull: {cnt.max()}"
    node_sorted = np.argsort(blk_of_node, kind="stable")
    starts = np.concatenate([[0], np.cumsum(cnt)[:-1]])
    slot_sorted = np.arange(n_nodes) - np.repeat(starts, cnt)
    slot_of_node = np.empty(n_nodes, np.int64)
    slot_of_node[node_sorted] = slot_sorted
    pos_of_node = blk_of_node * P + slot_of_node
    return pos_of_node, blk_of_node, slot_of_node


def build_edge_tables(src, dst, pos, blk, slot):
    """Uniform chunk grid + per-core offset/dloc tables."""
    dblk = blk[dst]
    core = dblk // BPC
    dslot = dblk % BPC
    spos = pos[src]

    key = core * BPC + dslot
    order = np.argsort(key, kind="stable")
    cnts = np.bincount(key, minlength=N_CORES * BPC)
    ch = np.ceil(cnts / P).astype(np.int64).reshape(N_CORES, BPC)
    cm = np.maximum(ch.max(axis=0), 1)              # [BPC]

    NCHTOT = int(cm.sum())
    starts = np.concatenate([[0], np.cumsum(cnts)[:-1]])
    chcol_of_slot = np.concatenate([[0], np.cumsum(cm)[:-1]])

    dst_sl = slot[dst].astype(np.int32)

    per_core = []
    for k in range(N_CORES):
        isrc = np.zeros((P, NCHTOT), np.int32)
        idstg = np.zeros((P, NCHTOT), np.int32)
        idstl = np.zeros((P, NCHTOT), np.int32)
        dloc = np.full((P, NCHTOT), PADLOC, np.float32)
        for s in range(BPC):
            ki = k * BPC + s
            n = cnts[ki]
            e = order[starts[ki]:starts[ki] + n]
            chcol = chcol_of_slot[s]
            cols = chcol + np.arange(n) // P
            rows = np.arange(n) % P
            isrc[rows, cols] = spos[e]
            idstg[rows, cols] = (k * BPC + s) * P + dst_sl[e]
            idstl[rows, cols] = s * P + dst_sl[e]
            dloc[rows, cols] = dst_sl[e]
        per_core.append({
            "ISRC": isrc,
            "IDSTG": idstg,
            "IDSTL": idstl,
            "DLOC": dloc.astype(ml_dtypes.bfloat16),
        })
    return cm, per_core


def build_inputs(x, edge_index, W1, a_src1, a_dst1, b1, W2, a_src2, a_dst2,
                 b2, n_cores):
    src = np.asarray(edge_index[0], dtype=np.int64)
    dst = np.asarray(edge_index[1], dtype=np.int64)
    pos, blk, slot = preprocess_graph(src, dst, N_NODES)
    cm, per_core = build_edge_tables(src, dst, pos, blk, slot)
    NCHG = cm.reshape(NGRP, G).sum(axis=1)
    KMAX = max(b - a for g in range(NGRP)
               for (a, b) in _ranges(int(NCHG[g]), NRQ))

    x = np.asarray(x, np.float32)
    XTa = np.zeros((IN_DIM, NPAD), np.float32)
    XTa[:, pos] = x.T

    W1 = np.asarray(W1, np.float32)
    W2 = np.asarray(W2, np.float32)
    a_src1 = np.asarray(a_src1, np.float32)
    a_dst1 = np.asarray(a_dst1, np.float32)
    a_src2 = np.asarray(a_src2, np.float32)
    a_dst2 = np.asarray(a_dst2, np.float32)
    b1 = np.asarray(b1, np.float32)
    b2 = np.asarray(b2, np.float32)

    # A1BD columns: [adst1(8) | asrc1(8)] per-head block-diagonal
    A1BD = np.zeros((HID, 2 * H1), np.float32)
    for h in range(H1):
        A1BD[h * C1:(h + 1) * C1, h] = a_dst1[h]
        A1BD[h * C1:(h + 1) * C1, H1 + h] = a_src1[h]
    W1AUG = np.concatenate([W1, W1 @ A1BD], axis=1)

    A2T = np.stack([a_dst2[0], a_src2[0]], axis=1)     # [OUT_DIM, 2]
    W2AUG = np.concatenate([W2 @ A2T, W2], axis=1)     # [HID, 2+OUT_DIM]

    iota = np.broadcast_to(np.arange(P, dtype=np.float32), (P, P))
    iotab = np.tile(iota, (1, KMAX))

    XTb = XTa.astype(ml_dtypes.bfloat16)
    common = {
        "W1AUG": W1AUG.astype(ml_dtypes.bfloat16),
        "W2AUG": W2AUG.astype(ml_dtypes.bfloat16),
        "B1R": np.ascontiguousarray(np.broadcast_to(b1, (P, HID))),
        "B2R": np.ascontiguousarray(np.broadcast_to(b2, (P, OUT_DIM))),
        "IOTAB": np.ascontiguousarray(iotab).astype(ml_dtypes.bfloat16),
        "IDENT": np.eye(P, dtype=ml_dtypes.bfloat16),
    }
    in_maps = []
    for k, pc in enumerate(per_core):
        m = dict(common, **pc)
        m["XTB"] = np.ascontiguousarray(
            XTb[:, k * BPC * P:(k + 1) * BPC * P])
        in_maps.append(m)
    return in_maps, pos, cm


# ----------------------------------------------------------------------------
# entry point
# ----------------------------------------------------------------------------

_prog_cache = {}
last_results = None


def _get_program(cm, n_cores):
    key = (cm.tobytes(), n_cores)
    if key not in _prog_cache:
        nc = build_program(cm, n_cores)
        _split_excess_waits(nc)
        _prog_cache[key] = nc
    return _prog_cache[key]


def run(inputs, n_cores=N_CORES, trace=False):
    global last_results
    in_maps, pos, cm = build_inputs(n_cores=n_cores, **inputs)
    nc = _get_program(cm, n_cores)
    kwargs = {}
    if trace:
        kwargs = dict(trace=True, trace_cores=[0])
    res = run_bass_kernel_spmd(
        nc, in_maps, core_ids=list(range(n_cores)), **kwargs)
    last_results = res
    out_all = np.concatenate([r["OUT"] for r in res.results], axis=0)
    return np.ascontiguousarray(out_all[pos].astype(np.float32))


def kernel(**inputs):
    return run(inputs)


# revision 19
# speedup vs baseline: 1.9039x; 1.7665x over previous
"""Trainium2 Bass kernel: 2-layer GAT (nn_GAT_1709396983866), v5.

Strategy (graph/data parallel over 8 NeuronCores):
  * Nodes are permuted and packed into 784 blocks of 128 positions, balanced
    by in-degree (~2041 edges/block). Core k owns blocks [98k, 98k+98); edges
    are sharded by destination block so segment-softmax / scatter-add stay
    core-local. Each block's edges form chunks of 128 (same dst block); the
    per-slot chunk counts are padded to a cross-core-uniform grid `cm` so all
    8 cores run one SPMD program.
  * Phase A is sharded: each core computes h / attention-alpha rows only for
    its own 12544 node positions from a per-core X slice (per-core input is
    ~6.5MB instead of ~53MB), then the T1/DSTT tables are AllGathered.
  * Per-edge feature gathers use one [128,1]-offset indirect_dma_start per
    chunk (multi-offset indirect mispairs offsets with descriptors on HW
    because the DGE consumes them in engine-spray order - probed empirically;
    dma_gather crashes under this runtime's Q7 library).
  * Gather tables:
      T1    [npad, 136] bf16: [asrc1(8) | h(128)]      (by src, layer 1)
      DSTT  [npad, 8]   bf16: adst1                    (by dst, layer 1)
      T2    [npad, 41]  f32:  [asrc2(1) | h2(40)]      (by src, layer 2)
      DSTT2 [12544, 4]  f32:  adst2       (core-local, by dst, layer 2)
  * Per chunk: one-hot mask[e,d] = (dst_local[e]==d); logits = asrc[src] +
    adst[dst] from the two gathers; exp(leaky_relu(x)) = max(exp(x),
    exp(0.2x)) (exact, by monotonicity); messages [h*exp | exp] are
    scatter-added per dst block via mask.T @ msg on the tensor engine (PSUM
    accumulation, one bank per block slot so start=True bank-clears can't
    clobber a sibling accumulation).
  * Blocks are processed in groups of 7, chunks in ranges of ~28; all
    elementwise work is batched per range (~3600 elems/partition per DVE
    instruction), which cuts DVE+ACT+scalar instruction time from ~5.8ms
    (baseline, per-chunk ops) to ~1.0ms. Remaining bottleneck: the 6272
    per-chunk indirect gathers at ~1.1-1.2us of serialized Q7 descriptor
    generation each (~7.5ms on the GpSimd engine).

kernel(**inputs) takes the full unsharded inputs and returns the full output.
"""

import numpy as np
import ml_dtypes

import concourse.bass as bass
import concourse.tile as tile
from concourse import mybir
from concourse.bass_utils import run_bass_kernel_spmd
from concourse.tile_rust import add_dep_helper


# Per-opcode embedded sync-wait slot budget in walrus codegen (empirical).
_WAIT_LIMITS = {}
_WAIT_LIMIT_DEFAULT = 1
_NOSPLIT_OPS = ("EventSemaphore",)


def _split_excess_waits(nc):
    """Move excess sem waits onto preceding same-engine wait instructions."""
    nid = [0]

    def mk_wait(engine, wait):
        nid[0] += 1
        ev = mybir.InstEventSemaphore(
            name=f"waitsplit-{nid[0]}", ins=[], outs=[])
        ev.engine = engine
        ev.sync_info = mybir.SyncInfo(on_wait=[wait], on_update=[])
        return ev

    for fn in nc.m.functions:
        for bb in fn.blocks:
            out = []
            for inst in bb.instructions:
                si = inst.sync_info
                waits = list(si.on_wait) if si and si.on_wait else []
                lim = _WAIT_LIMITS.get(inst.opcode, _WAIT_LIMIT_DEFAULT)
                if len(waits) > lim and inst.opcode not in _NOSPLIT_OPS:
                    excess, keep = waits[:-lim], waits[-lim:]
                    for w in excess:
                        out.append(mk_wait(inst.engine, w))
                    inst.sync_info = mybir.SyncInfo(
                        on_wait=keep, on_update=list(si.on_update or []))
                out.append(inst)
            bb.instructions = out


def _phase_barrier(tc, nc):
    """All-engine barrier that soaks per-DMA-lane waits per engine first."""
    curr_bb = nc.cur_bb
    prev = list(curr_bb.bb.instructions)
    for eng in (nc.gpsimd, nc.sync, nc.scalar, nc.vector, nc.tensor):
        nop = eng.nop()
        for inst in prev:
            add_dep_helper(
                nop.ins, inst,
                sync=bass.sync_unless_reorderable_target(
                    inst, inst.is_executable()),
                reason="phase-barrier soak")
    tc.strict_bb_all_engine_barrier()


# -------- problem constants (hardcoded, per spec) --------
N_NODES = 100000
IN_DIM = 128
HID = 128
OUT_DIM = 40
H1 = 8
C1 = 16
NEG_SLOPE = 0.2
EPS = 1e-16
DENOM_FLOOR = 1e-6
N_CORES = 8
P = 128
NBLK = 784
BPC = NBLK // N_CORES          # 98
NPAD = NBLK * P                # 100352
G = 7                          # block slots per group
NGRP = BPC // G                # 14
GROWS = G * P                  # 896 rows per group
PADLOC = 200.0                 # dst_local for padding edge slots
TPB = 2                        # blocks per phase-A batch
NRQ = 4                        # chunk ranges per group (gather call batches)

T1W = H1 + HID                 # 136: [asrc1 | h]
DSTW = H1                      # 8: adst1
T2W = 1 + OUT_DIM              # 41: [asrc2 | h2]
ACC1W = HID + H1               # 136
ACC2W = OUT_DIM + 1            # 41

F32 = mybir.dt.float32
BF16 = mybir.dt.bfloat16
I32 = mybir.dt.int32
AF = mybir.ActivationFunctionType
OP = mybir.AluOpType


def _ranges(n, q):
    """Split range(n) into q near-equal contiguous pieces."""
    out = []
    a = 0
    for i in range(q):
        b = a + (n - a) // (q - i)
        if b > a:
            out.append((a, b))
        a = b
    return out


def build_program(cm, n_cores):
    """cm: [BPC] uniform per-slot chunk counts (shared by all cores)."""
    cm = np.asarray(cm)
    NCHS = cm.reshape(NGRP, G)                      # [group, slot]
    NCHG = NCHS.sum(axis=1)                         # chunks per group
    NCHTOT = int(NCHG.sum())
    grp_ch0 = np.concatenate([[0], np.cumsum(NCHG)[:-1]]).astype(int)
    KMAX = max(b - a for g in range(NGRP)
               for (a, b) in _ranges(int(NCHG[g]), NRQ))

    nc = bass.Bass(num_devices=n_cores)

    # ---------------- I/O ----------------
    XTB = nc.dram_tensor("XTB", [IN_DIM, BPC * P], BF16,
                          kind="ExternalInput")
    W1AUGd = nc.dram_tensor("W1AUG", [IN_DIM, HID + 2 * H1], BF16,
                            kind="ExternalInput")
    W2AUGd = nc.dram_tensor("W2AUG", [HID, 2 + OUT_DIM], BF16,
                            kind="ExternalInput")
    B1Rd = nc.dram_tensor("B1R", [P, HID], F32, kind="ExternalInput")
    B2Rd = nc.dram_tensor("B2R", [P, OUT_DIM], F32, kind="ExternalInput")
    IOTABd = nc.dram_tensor("IOTAB", [P, KMAX * P], BF16,
                            kind="ExternalInput")
    IDENTd = nc.dram_tensor("IDENT", [P, P], BF16, kind="ExternalInput")
    ISRCd = nc.dram_tensor("ISRC", [P, NCHTOT], I32, kind="ExternalInput")
    IDST0d = nc.dram_tensor("IDST0", [P, BPC], I32, kind="ExternalInput")
    DLOCd = nc.dram_tensor("DLOC", [P, NCHTOT], BF16, kind="ExternalInput")
    OUTd = nc.dram_tensor("OUT", [BPC * P, OUT_DIM], F32,
                          kind="ExternalOutput")

    # ---------------- internal DRAM ----------------
    T1Ld = nc.dram_tensor("T1L", [BPC * P, T1W], BF16)
    DSTTLd = nc.dram_tensor("DSTTL", [BPC * P, DSTW], BF16)
    T1d = nc.dram_tensor("T1", [NPAD, T1W], BF16, addr_space="Shared")
    DSTTd = nc.dram_tensor("DSTT", [NPAD, DSTW], BF16, addr_space="Shared")
    DSTT2d = nc.dram_tensor("DSTT2", [BPC * P, 4], F32)
    T2Ld = nc.dram_tensor("T2L", [BPC * P, T2W], F32)
    T2d = nc.dram_tensor("T2", [NPAD, T2W], F32, addr_space="Shared")

    with tile.TileContext(nc) as tc:
        with tc.tile_pool(name="consts", bufs=1) as cp:
            W1AUG_sb = cp.tile([IN_DIM, HID + 2 * H1], BF16)
            nc.sync.dma_start(out=W1AUG_sb[:], in_=W1AUGd[:, :])
            W2AUG_sb = cp.tile([HID, 2 + OUT_DIM], BF16)
            nc.sync.dma_start(out=W2AUG_sb[:], in_=W2AUGd[:, :])
            B1R_sb = cp.tile([P, HID], F32)
            nc.sync.dma_start(out=B1R_sb[:], in_=B1Rd[:, :])
            B2R_sb = cp.tile([P, OUT_DIM], F32)
            nc.sync.dma_start(out=B2R_sb[:], in_=B2Rd[:, :])
            IOTAB_sb = cp.tile([P, KMAX * P], BF16)
            nc.sync.dma_start(out=IOTAB_sb[:], in_=IOTABd[:, :])
            IDENT_sb = cp.tile([P, P], BF16)
            nc.sync.dma_start(out=IDENT_sb[:], in_=IDENTd[:, :])
            DLOC_sb = cp.tile([P, NCHTOT], BF16)
            nc.sync.dma_start(out=DLOC_sb[:], in_=DLOCd[:, :])
            ISRC_sb = cp.tile([P, NCHTOT], I32)
            nc.sync.dma_start(out=ISRC_sb[:], in_=ISRCd[:, :])
            IDST0_sb = cp.tile([P, BPC], I32)
            nc.sync.dma_start(out=IDST0_sb[:], in_=IDST0d[:, :])

            # ================= Phase A: T1 / DSTT generation ================
            with tc.tile_pool(name="pa", bufs=3) as pa, \
                 tc.tile_pool(name="papsum", bufs=2, space="PSUM") as pap:
                for tb in range(BPC // TPB):
                    c0 = tb * TPB * P
                    xt = pa.tile([IN_DIM, TPB * P], BF16, tag="xt")
                    nc.sync.dma_start(out=xt[:], in_=XTB[:, c0:c0 + TPB * P])
                    # block i at a 2KB-aligned 512-f32 stride so each matmul
                    # output sits in one PSUM bank
                    hal = pap.tile([P, TPB * 512], F32, tag="hal")
                    hal4 = hal[:].rearrange("p (t c) -> p t c", c=512)
                    for i in range(TPB):
                        nc.tensor.matmul(
                            hal[:, i * 512:i * 512 + 144],
                            lhsT=xt[:, i * P:(i + 1) * P], rhs=W1AUG_sb[:],
                            start=True, stop=True)
                    t1 = pa.tile([P, TPB, T1W], BF16, tag="t1")
                    nc.vector.tensor_copy(out=t1[:, :, 8:136],
                                          in_=hal4[:, :, 0:128])
                    nc.vector.tensor_copy(out=t1[:, :, 0:8],
                                          in_=hal4[:, :, 136:144])
                    dstt = pa.tile([P, TPB, DSTW], BF16, tag="dstt")
                    nc.vector.tensor_copy(out=dstt[:, :, 0:8],
                                          in_=hal4[:, :, 128:136])
                    nc.sync.dma_start(
                        out=T1Ld[c0:c0 + TPB * P, :].rearrange(
                            "(t p) e -> p t e", p=P),
                        in_=t1[:])
                    nc.scalar.dma_start(
                        out=DSTTLd[c0:c0 + TPB * P, :].rearrange(
                            "(t p) e -> p t e", p=P),
                        in_=dstt[:])

            # AllGather the phase-A table shards
            nc.gpsimd.collective_compute(
                "AllGather", OP.bypass,
                replica_groups=[list(range(n_cores))],
                ins=[T1Ld[:, :].opt()],
                outs=[T1d[:, :].opt()],
            )
            nc.gpsimd.collective_compute(
                "AllGather", OP.bypass,
                replica_groups=[list(range(n_cores))],
                ins=[DSTTLd[:, :].opt()],
                outs=[DSTTd[:, :].opt()],
            )

            _phase_barrier(tc, nc)

            # ============ Phase B/C: edge processing (shared shape) =========
            def edge_phase(layer):
                sdt = BF16 if layer == 1 else F32
                srcw = T1W if layer == 1 else T2W
                srcwp = 144 if layer == 1 else 48   # padded tile stride
                dstw = DSTW if layer == 1 else 4
                dstwp = 16 if layer == 1 else 8     # padded tile stride
                accw = ACC1W if layer == 1 else ACC2W
                nhd = H1 if layer == 1 else 1
                fdim = HID if layer == 1 else OUT_DIM
                srcT = T1d if layer == 1 else T2d
                tg = f"L{layer}"

                with tc.tile_pool(name=f"pg{layer}", bufs=3) as pg, \
                     tc.tile_pool(name=f"pm{layer}", bufs=2) as pm, \
                     tc.tile_pool(name=f"pe{layer}", bufs=2) as pe, \
                     tc.tile_pool(name=f"pp{layer}", bufs=1,
                                  space="PSUM") as pp, \
                     tc.tile_pool(name=f"ppm{layer}", bufs=2,
                                  space="PSUM") as ppm, \
                     tc.tile_pool(name=f"ppq{layer}", bufs=1,
                                  space="PSUM") as ppq, \
                     tc.tile_pool(name=f"pq{layer}", bufs=1,
                                  space="PSUM") as pq:
                    for g in range(NGRP):
                        ch0g = int(grp_ch0[g])
                        nchg = int(NCHG[g])
                        # chunk -> (slot-in-group, chunk-in-slot)
                        sl_of = []
                        for s7 in range(G):
                            for c in range(int(NCHS[g, s7])):
                                sl_of.append((s7, c))
                        # per-slot acc regions at a 1KB stride (two slots
                        # per bank; safe: a slot's accumulation group fully
                        # precedes its bank-sibling's start, and start=True
                        # only clears accumulate bits, not data)
                        accT = pp.tile([P, G * 256], F32, tag="acc",
                                       name=f"acc{layer}_{g}")
                        # per-group adst tiles (replaces per-chunk dst
                        # gathers: 2x fewer indirect calls)
                        if layer == 1:
                            adstg = pg.tile([P, G, DSTW], BF16, tag="adstg")
                            for s7 in range(G):
                                nc.gpsimd.indirect_dma_start(
                                    out=adstg[:, s7, :], out_offset=None,
                                    in_=DSTTd[:, :],
                                    in_offset=bass.IndirectOffsetOnAxis(
                                        ap=IDST0_sb[:, g * G + s7:
                                                    g * G + s7 + 1],
                                        axis=0))
                        else:
                            # SWDGE cast f32 -> bf16 during the load (the PE
                            # rejects mixed f32/bf16 matmul operands)
                            adstg = pg.tile([P, G, 4], BF16, tag="adstg2")
                            nc.gpsimd.dma_start(
                                out=adstg[:],
                                in_=DSTT2d[g * GROWS:(g + 1) * GROWS, :]
                                    .rearrange("(t p) e -> p t e", p=P))
                        for (a, b) in _ranges(nchg, NRQ):
                            k = b - a
                            c0 = ch0g + a
                            # one [128,1]-offset indirect per chunk (the
                            # multi-offset form mispairs offsets on HW)
                            gt = pg.tile([P, KMAX, srcwp], sdt,
                                         tag=f"gt{tg}")
                            for j in range(k):
                                nc.gpsimd.indirect_dma_start(
                                    out=gt[:, j, 0:srcw], out_offset=None,
                                    in_=srcT[:, :],
                                    in_offset=bass.IndirectOffsetOnAxis(
                                        ap=ISRC_sb[:, c0 + j:c0 + j + 1],
                                        axis=0))

                            # ---- batched edge compute for this range ----
                            mask = pm.tile([P, KMAX, P], BF16,
                                           tag=f"mask{tg}")
                            nc.vector.tensor_tensor(
                                out=mask[:, 0:k, :],
                                in0=IOTAB_sb[:, 0:k * P].rearrange(
                                    "p (n d) -> p n d", d=P),
                                in1=DLOC_sb[:, c0:c0 + k]
                                    .unsqueeze(2).to_broadcast([P, k, P]),
                                op=OP.is_equal)
                            # maskT (PE transposes, batched PSUM->SBUF
                            # copies), then per-edge adst = maskT.T @
                            # adst_block on the tensor engine
                            mts = pm.tile([P, KMAX, P], BF16,
                                          tag=f"mts{tg}")
                            for j0 in range(0, k, 8):
                                jn = min(8, k - j0)
                                mt_ps = ppm.tile([P, 8, P], BF16,
                                                 tag="mtps",
                                                 name=f"mtps{layer}_{g}_"
                                                      f"{c0}_{j0}")
                                for j in range(j0, j0 + jn):
                                    nc.tensor.transpose(
                                        mt_ps[:, j - j0, :],
                                        mask[:, j, :], IDENT_sb[:])
                                nc.vector.tensor_copy(
                                    out=mts[:, j0:j0 + jn, :],
                                    in_=mt_ps[:, 0:jn, :])
                            adps = ppq.tile([P, 512], F32, tag="adps",
                                            name=f"adps{layer}_{g}_{c0}")
                            apv = adps[:, 0:k * nhd].rearrange(
                                "p (n h) -> p n h", h=nhd)
                            for j in range(k):
                                s7j = sl_of[a + j][0]
                                nc.tensor.matmul(
                                    apv[:, j, :],
                                    lhsT=mts[:, j, :],
                                    rhs=adstg[:, s7j, 0:nhd],
                                    start=True, stop=True,
                                    skip_group_check=True)
                            lg = pm.tile([P, KMAX, nhd], F32, tag=f"lg{tg}")
                            nc.vector.tensor_tensor(
                                out=lg[:, 0:k, :],
                                in0=gt[:, 0:k, 0:nhd],
                                in1=apv[:, 0:k, :], op=OP.add)
                            e1 = pm.tile([P, KMAX, nhd], F32, tag=f"e1{tg}")
                            nc.scalar.activation(out=e1[:, 0:k, :],
                                                 in_=lg[:, 0:k, :],
                                                 func=AF.Exp)
                            e2 = pm.tile([P, KMAX, nhd], F32, tag=f"e2{tg}")
                            nc.scalar.activation(out=e2[:, 0:k, :],
                                                 in_=lg[:, 0:k, :],
                                                 func=AF.Exp, scale=NEG_SLOPE)
                            msg = pm.tile([P, KMAX, accw], BF16,
                                          tag=f"msg{tg}")
                            nc.vector.tensor_tensor(
                                out=msg[:, 0:k, fdim:accw],
                                in0=e1[:, 0:k, :], in1=e2[:, 0:k, :],
                                op=OP.max)
                            if layer == 1:
                                m4 = msg[:, 0:k, 0:fdim].rearrange(
                                    "p n (h c) -> p n h c", c=C1)
                                h4 = gt[:, 0:k, 8:136].rearrange(
                                    "p n (h c) -> p n h c", c=C1)
                                x4 = msg[:, 0:k, fdim:accw].unsqueeze(3) \
                                    .to_broadcast([P, k, H1, C1])
                                nc.vector.tensor_tensor(
                                    out=m4, in0=h4, in1=x4, op=OP.mult)
                            else:
                                nc.vector.tensor_tensor(
                                    out=msg[:, 0:k, 0:fdim],
                                    in0=gt[:, 0:k, 1:1 + OUT_DIM],
                                    in1=msg[:, 0:k, fdim:accw]
                                        .to_broadcast([P, k, OUT_DIM]),
                                    op=OP.mult)

                            # ---- scatter-accumulate per chunk ----
                            for j in range(a, b):
                                s7, c = sl_of[j]
                                nc.tensor.matmul(
                                    accT[:, s7 * 256:s7 * 256 + accw],
                                    lhsT=mask[:, j - a, :],
                                    rhs=msg[:, j - a, :],
                                    start=(c == 0),
                                    stop=(c == int(NCHS[g, s7]) - 1),
                                    skip_group_check=True)

                        # ---- group epilogue ----
                        accsb = pe.tile([P, G, accw], F32, tag=f"accsb{tg}")
                        for s7 in range(G):
                            nc.vector.tensor_copy(
                                out=accsb[:, s7, :],
                                in_=accT[:, s7 * 256:s7 * 256 + accw])
                        if layer == 1:
                            dinv = pe.tile([P, G, H1], F32, tag="dinv")
                            nc.vector.tensor_scalar(
                                out=dinv[:], in0=accsb[:, :, HID:ACC1W],
                                scalar1=EPS, scalar2=DENOM_FLOOR,
                                op0=OP.add, op1=OP.max)
                            nc.vector.reciprocal(out=dinv[:], in_=dinv[:])
                            h1 = pe.tile([P, G, HID], F32, tag="h1")
                            a4 = accsb[:, :, 0:HID].rearrange(
                                "p g (h c) -> p g h c", c=C1)
                            dv4 = dinv[:].unsqueeze(3).to_broadcast(
                                [P, G, H1, C1])
                            h14 = h1[:].rearrange(
                                "p g (h c) -> p g h c", c=C1)
                            nc.vector.tensor_tensor(
                                out=h14, in0=a4, in1=dv4, op=OP.mult)
                            nc.vector.tensor_tensor(
                                out=h1[:], in0=h1[:],
                                in1=B1R_sb[:].unsqueeze(1)
                                    .to_broadcast([P, G, HID]),
                                op=OP.add)
                            # ELU(x) = max(x,0) + min(exp(x)-1, 0)
                            ex = pe.tile([P, G, HID], F32, tag="ex")
                            nc.scalar.activation(out=ex[:], in_=h1[:],
                                                 func=AF.Exp)
                            nc.vector.tensor_scalar(
                                out=ex[:], in0=ex[:], scalar1=-1.0,
                                scalar2=0.0, op0=OP.add, op1=OP.min)
                            nc.vector.tensor_scalar(
                                out=h1[:], in0=h1[:], scalar1=0.0,
                                scalar2=None, op0=OP.max)
                            h1e = pe.tile([P, G, HID], BF16, tag="h1e")
                            nc.vector.tensor_tensor(out=h1e[:], in0=h1[:],
                                                    in1=ex[:], op=OP.add)
                            for s7 in range(G):
                                s = g * G + s7
                                epi = pq.tile([P, 1024], BF16, tag="epi",
                                              name=f"epi_{g}_{s7}")
                                h1T_ps = epi[:, 0:HID]
                                nc.tensor.transpose(h1T_ps,
                                                    h1e[:, s7, :],
                                                    IDENT_sb[:])
                                h1T = pe.tile([P, HID], BF16, tag="h1Ts")
                                nc.vector.tensor_copy(out=h1T[:],
                                                      in_=h1T_ps)
                                hg2x = epi[:, 256:256 + 2 * (2 + OUT_DIM)]
                                hg2 = hg2x.bitcast(F32)
                                nc.tensor.matmul(hg2, lhsT=h1T[:],
                                                 rhs=W2AUG_sb[:],
                                                 start=True, stop=True)
                                t2row = pe.tile([P, T2W], F32, tag="t2r")
                                nc.vector.tensor_copy(
                                    out=t2row[:],
                                    in_=hg2[:, 1:2 + OUT_DIM])
                                d2row = pe.tile([P, 4], F32, tag="d2r")
                                nc.vector.tensor_copy(out=d2row[:, 0:1],
                                                      in_=hg2[:, 0:1])
                                nc.scalar.dma_start(
                                    out=T2Ld[s * P:(s + 1) * P, :],
                                    in_=t2row[:])
                                nc.scalar.dma_start(
                                    out=DSTT2d[s * P:(s + 1) * P, :],
                                    in_=d2row[:])
                        else:
                            dinv2 = pe.tile([P, G, 1], F32, tag="dinv2")
                            nc.vector.tensor_scalar(
                                out=dinv2[:], in0=accsb[:, :, OUT_DIM:ACC2W],
                                scalar1=EPS, scalar2=DENOM_FLOOR,
                                op0=OP.add, op1=OP.max)
                            nc.vector.reciprocal(out=dinv2[:], in_=dinv2[:])
                            o = pe.tile([P, G, OUT_DIM], F32, tag="o")
                            nc.vector.tensor_tensor(
                                out=o[:], in0=accsb[:, :, 0:OUT_DIM],
                                in1=dinv2[:].to_broadcast([P, G, OUT_DIM]),
                                op=OP.mult)
                            nc.vector.tensor_tensor(
                                out=o[:], in0=o[:],
                                in1=B2R_sb[:].unsqueeze(1)
                                    .to_broadcast([P, G, OUT_DIM]),
                                op=OP.add)
                            # log_softmax = (o - m) - ln(sum(exp(o - m)))
                            nm = pe.tile([P, G, 1], F32, tag="nm")
                            nc.vector.tensor_reduce(
                                out=nm[:], in_=o[:],
                                axis=mybir.AxisListType.X,
                                op=OP.max, negate=True)
                            osh = pe.tile([P, G, OUT_DIM], F32, tag="osh")
                            nc.vector.tensor_tensor(
                                out=osh[:], in0=o[:],
                                in1=nm[:].to_broadcast([P, G, OUT_DIM]),
                                op=OP.add)
                            e2t = pe.tile([P, G, OUT_DIM], F32, tag="e2t")
                            nc.scalar.activation(out=e2t[:], in_=osh[:],
                                                 func=AF.Exp)
                            s2 = pe.tile([P, G, 1], F32, tag="s2")
                            nc.vector.tensor_reduce(
                                out=s2[:], in_=e2t[:],
                                axis=mybir.AxisListType.X, op=OP.add)
                            ls = pe.tile([P, G, 1], F32, tag="ls")
                            nc.scalar.activation(out=ls[:], in_=s2[:],
                                                 func=AF.Ln)
                            ot = pe.tile([P, G, OUT_DIM], F32, tag="ot")
                            nc.vector.tensor_tensor(
                                out=ot[:], in0=osh[:],
                                in1=ls[:].to_broadcast([P, G, OUT_DIM]),
                                op=OP.subtract)
                            nc.sync.dma_start(
                                out=OUTd[g * GROWS:(g + 1) * GROWS, :]
                                    .rearrange("(t p) e -> p t e", p=P),
                                in_=ot[:])

            edge_phase(1)

            # ============= AllGather of T2 shards ===============
            nc.gpsimd.collective_compute(
                "AllGather", OP.bypass,
                replica_groups=[list(range(n_cores))],
                ins=[T2Ld[:, :].opt()],
                outs=[T2d[:, :].opt()],
            )
            _phase_barrier(tc, nc)

            edge_phase(2)

    return nc


# ----------------------------------------------------------------------------
# host-side preprocessing (index/layout work)
# ----------------------------------------------------------------------------

def preprocess_graph(src, dst, n_nodes):
    """Assign nodes to in-degree-balanced blocks of 128."""
    deg = np.bincount(dst, minlength=n_nodes)
    order = np.argsort(-deg, kind="stable")
    r = np.arange(n_nodes)
    rounds, posr = r // NBLK, r % NBLK
    binr = np.where(rounds % 2 == 0, posr, NBLK - 1 - posr)
    blk_of_node = np.empty(n_nodes, np.int64)
    blk_of_node[order] = binr
    cnt = np.bincount(blk_of_node, minlength=NBLK)
    assert cnt.max() <= P, f"block overfull: {cnt.max()}"
    node_sorted = np.argsort(blk_of_node, kind="stable")
    starts = np.concatenate([[0], np.cumsum(cnt)[:-1]])
    slot_sorted = np.arange(n_nodes) - np.repeat(starts, cnt)
    slot_of_node = np.empty(n_nodes, np.int64)
    slot_of_node[node_sorted] = slot_sorted
    pos_of_node = blk_of_node * P + slot_of_node
    return pos_of_node, blk_of_node, slot_of_node


def build_edge_tables(src, dst, pos, blk, slot):
    """Uniform chunk grid + per-core offset/dloc tables."""
    dblk = blk[dst]
    core = dblk // BPC
    dslot = dblk % BPC
    spos = pos[src]

    key = core * BPC + dslot
    order = np.argsort(key, kind="stable")
    cnts = np.bincount(key, minlength=N_CORES * BPC)
    ch = np.ceil(cnts / P).astype(np.int64).reshape(N_CORES, BPC)
    cm = np.maximum(ch.max(axis=0), 1)              # [BPC]

    NCHTOT = int(cm.sum())
    starts = np.concatenate([[0], np.cumsum(cnts)[:-1]])
    chcol_of_slot = np.concatenate([[0], np.cumsum(cm)[:-1]])

    dst_sl = slot[dst].astype(np.int32)

    per_core = []
    for k in range(N_CORES):
        isrc = np.zeros((P, NCHTOT), np.int32)
        dloc = np.full((P, NCHTOT), PADLOC, np.float32)
        for s in range(BPC):
            ki = k * BPC + s
            n = cnts[ki]
            e = order[starts[ki]:starts[ki] + n]
            chcol = chcol_of_slot[s]
            cols = chcol + np.arange(n) // P
            rows = np.arange(n) % P
            isrc[rows, cols] = spos[e]
            dloc[rows, cols] = dst_sl[e]
        # identity row offsets of each owned block (for per-group adst loads)
        idst0 = ((k * BPC + np.arange(BPC))[None, :] * P
                 + np.arange(P)[:, None]).astype(np.int32)
        per_core.append({
            "ISRC": isrc,
            "IDST0": idst0,
            "DLOC": dloc.astype(ml_dtypes.bfloat16),
        })
    return cm, per_core


def build_inputs(x, edge_index, W1, a_src1, a_dst1, b1, W2, a_src2, a_dst2,
                 b2, n_cores):
    src = np.asarray(edge_index[0], dtype=np.int64)
    dst = np.asarray(edge_index[1], dtype=np.int64)
    pos, blk, slot = preprocess_graph(src, dst, N_NODES)
    cm, per_core = build_edge_tables(src, dst, pos, blk, slot)
    NCHG = cm.reshape(NGRP, G).sum(axis=1)
    KMAX = max(b - a for g in range(NGRP)
               for (a, b) in _ranges(int(NCHG[g]), NRQ))

    x = np.asarray(x, np.float32)
    XTa = np.zeros((IN_DIM, NPAD), np.float32)
    XTa[:, pos] = x.T

    W1 = np.asarray(W1, np.float32)
    W2 = np.asarray(W2, np.float32)
    a_src1 = np.asarray(a_src1, np.float32)
    a_dst1 = np.asarray(a_dst1, np.float32)
    a_src2 = np.asarray(a_src2, np.float32)
    a_dst2 = np.asarray(a_dst2, np.float32)
    b1 = np.asarray(b1, np.float32)
    b2 = np.asarray(b2, np.float32)

    # A1BD columns: [adst1(8) | asrc1(8)] per-head block-diagonal
    A1BD = np.zeros((HID, 2 * H1), np.float32)
    for h in range(H1):
        A1BD[h * C1:(h + 1) * C1, h] = a_dst1[h]
        A1BD[h * C1:(h + 1) * C1, H1 + h] = a_src1[h]
    W1AUG = np.concatenate([W1, W1 @ A1BD], axis=1)

    A2T = np.stack([a_dst2[0], a_src2[0]], axis=1)     # [OUT_DIM, 2]
    W2AUG = np.concatenate([W2 @ A2T, W2], axis=1)     # [HID, 2+OUT_DIM]

    iota = np.broadcast_to(np.arange(P, dtype=np.float32), (P, P))
    iotab = np.tile(iota, (1, KMAX))

    XTb = XTa.astype(ml_dtypes.bfloat16)
    common = {
        "W1AUG": W1AUG.astype(ml_dtypes.bfloat16),
        "W2AUG": W2AUG.astype(ml_dtypes.bfloat16),
        "B1R": np.ascontiguousarray(np.broadcast_to(b1, (P, HID))),
        "B2R": np.ascontiguousarray(np.broadcast_to(b2, (P, OUT_DIM))),
        "IOTAB": np.ascontiguousarray(iotab).astype(ml_dtypes.bfloat16),
        "IDENT": np.eye(P, dtype=ml_dtypes.bfloat16),
    }
    in_maps = []
    for k, pc in enumerate(per_core):
        m = dict(common, **pc)
        m["XTB"] = np.ascontiguousarray(
            XTb[:, k * BPC * P:(k + 1) * BPC * P])
        in_maps.append(m)
    return in_maps, pos, cm


# ----------------------------------------------------------------------------
# entry point
# ----------------------------------------------------------------------------

_prog_cache = {}
last_results = None


def _get_program(cm, n_cores):
    key = (cm.tobytes(), n_cores)
    if key not in _prog_cache:
        nc = build_program(cm, n_cores)
        _split_excess_waits(nc)
        _prog_cache[key] = nc
    return _prog_cache[key]


def run(inputs, n_cores=N_CORES, trace=False):
    global last_results
    in_maps, pos, cm = build_inputs(n_cores=n_cores, **inputs)
    nc = _get_program(cm, n_cores)
    kwargs = {}
    if trace:
        kwargs = dict(trace=True, trace_cores=[0])
    res = run_bass_kernel_spmd(
        nc, in_maps, core_ids=list(range(n_cores)), **kwargs)
    last_results = res
    out_all = np.concatenate([r["OUT"] for r in res.results], axis=0)
    return np.ascontiguousarray(out_all[pos].astype(np.float32))


def kernel(**inputs):
    return run(inputs)


# revision 20
# speedup vs baseline: 1.9041x; 1.0001x over previous
"""Trainium2 Bass kernel: 2-layer GAT (nn_GAT_1709396983866), v5.

Strategy (graph/data parallel over 8 NeuronCores):
  * Nodes are permuted and packed into 784 blocks of 128 positions, balanced
    by in-degree (~2041 edges/block). Core k owns blocks [98k, 98k+98); edges
    are sharded by destination block so segment-softmax / scatter-add stay
    core-local. Each block's edges form chunks of 128 (same dst block); the
    per-slot chunk counts are padded to a cross-core-uniform grid `cm` so all
    8 cores run one SPMD program.
  * Phase A is sharded: each core computes h / attention-alpha rows only for
    its own 12544 node positions from a per-core X slice (per-core input is
    ~6.5MB instead of ~53MB), then the T1/DSTT tables are AllGathered.
  * Per-edge feature gathers use one [128,1]-offset indirect_dma_start per
    chunk (multi-offset indirect mispairs offsets with descriptors on HW
    because the DGE consumes them in engine-spray order - probed empirically;
    dma_gather crashes under this runtime's Q7 library).
  * Gather tables:
      T1    [npad, 136] bf16: [asrc1(8) | h(128)]      (by src, layer 1)
      DSTT  [npad, 8]   bf16: adst1                    (by dst, layer 1)
      T2    [npad, 41]  f32:  [asrc2(1) | h2(40)]      (by src, layer 2)
      DSTT2 [12544, 4]  f32:  adst2       (core-local, by dst, layer 2)
  * Per chunk: one-hot mask[e,d] = (dst_local[e]==d); logits = asrc[src] +
    adst[dst] from the two gathers; exp(leaky_relu(x)) = max(exp(x),
    exp(0.2x)) (exact, by monotonicity); messages [h*exp | exp] are
    scatter-added per dst block via mask.T @ msg on the tensor engine (PSUM
    accumulation, one bank per block slot so start=True bank-clears can't
    clobber a sibling accumulation).
  * Blocks are processed in groups of 7, chunks in ranges of ~28; all
    elementwise work is batched per range (~3600 elems/partition per DVE
    instruction), which cuts DVE+ACT+scalar instruction time from ~5.8ms
    (baseline, per-chunk ops) to ~1.0ms. Remaining bottleneck: the 6272
    per-chunk indirect gathers at ~1.1-1.2us of serialized Q7 descriptor
    generation each (~7.5ms on the GpSimd engine).

kernel(**inputs) takes the full unsharded inputs and returns the full output.
"""

import numpy as np
import ml_dtypes

import concourse.bass as bass
import concourse.tile as tile
from concourse import mybir
from concourse.bass_utils import run_bass_kernel_spmd
from concourse.tile_rust import add_dep_helper


# Per-opcode embedded sync-wait slot budget in walrus codegen (empirical).
_WAIT_LIMITS = {}
_WAIT_LIMIT_DEFAULT = 1
_NOSPLIT_OPS = ("EventSemaphore",)


def _split_excess_waits(nc):
    """Move excess sem waits onto preceding same-engine wait instructions."""
    nid = [0]

    def mk_wait(engine, wait):
        nid[0] += 1
        ev = mybir.InstEventSemaphore(
            name=f"waitsplit-{nid[0]}", ins=[], outs=[])
        ev.engine = engine
        ev.sync_info = mybir.SyncInfo(on_wait=[wait], on_update=[])
        return ev

    for fn in nc.m.functions:
        for bb in fn.blocks:
            out = []
            for inst in bb.instructions:
                si = inst.sync_info
                waits = list(si.on_wait) if si and si.on_wait else []
                lim = _WAIT_LIMITS.get(inst.opcode, _WAIT_LIMIT_DEFAULT)
                if len(waits) > lim and inst.opcode not in _NOSPLIT_OPS:
                    excess, keep = waits[:-lim], waits[-lim:]
                    for w in excess:
                        out.append(mk_wait(inst.engine, w))
                    inst.sync_info = mybir.SyncInfo(
                        on_wait=keep, on_update=list(si.on_update or []))
                out.append(inst)
            bb.instructions = out


def _phase_barrier(tc, nc):
    """All-engine barrier that soaks per-DMA-lane waits per engine first."""
    curr_bb = nc.cur_bb
    prev = list(curr_bb.bb.instructions)
    for eng in (nc.gpsimd, nc.sync, nc.scalar, nc.vector, nc.tensor):
        nop = eng.nop()
        for inst in prev:
            add_dep_helper(
                nop.ins, inst,
                sync=bass.sync_unless_reorderable_target(
                    inst, inst.is_executable()),
                reason="phase-barrier soak")
    tc.strict_bb_all_engine_barrier()


# -------- problem constants (hardcoded, per spec) --------
N_NODES = 100000
IN_DIM = 128
HID = 128
OUT_DIM = 40
H1 = 8
C1 = 16
NEG_SLOPE = 0.2
EPS = 1e-16
DENOM_FLOOR = 1e-6
N_CORES = 8
P = 128
NBLK = 784
BPC = NBLK // N_CORES          # 98
NPAD = NBLK * P                # 100352
G = 7                          # block slots per group
NGRP = BPC // G                # 14
GROWS = G * P                  # 896 rows per group
PADLOC = 200.0                 # dst_local for padding edge slots
TPB = 2                        # blocks per phase-A batch
NRQ = 4                        # chunk ranges per group (gather call batches)

T1W = H1 + HID                 # 136: [asrc1 | h]
DSTW = H1                      # 8: adst1
T2W = 1 + OUT_DIM              # 41: [asrc2 | h2]
ACC1W = HID + H1               # 136
ACC2W = OUT_DIM + 1            # 41

F32 = mybir.dt.float32
BF16 = mybir.dt.bfloat16
I32 = mybir.dt.int32
AF = mybir.ActivationFunctionType
OP = mybir.AluOpType


def _ranges(n, q):
    """Split range(n) into q near-equal contiguous pieces."""
    out = []
    a = 0
    for i in range(q):
        b = a + (n - a) // (q - i)
        if b > a:
            out.append((a, b))
        a = b
    return out


def build_program(cm, n_cores):
    """cm: [BPC] uniform per-slot chunk counts (shared by all cores)."""
    cm = np.asarray(cm)
    NCHS = cm.reshape(NGRP, G)                      # [group, slot]
    NCHG = NCHS.sum(axis=1)                         # chunks per group
    NCHTOT = int(NCHG.sum())
    grp_ch0 = np.concatenate([[0], np.cumsum(NCHG)[:-1]]).astype(int)
    KMAX = max(b - a for g in range(NGRP)
               for (a, b) in _ranges(int(NCHG[g]), NRQ))

    nc = bass.Bass(num_devices=n_cores,
                   dynamic_dma_scratch_size=32768)

    # ---------------- I/O ----------------
    XTB = nc.dram_tensor("XTB", [IN_DIM, BPC * P], BF16,
                          kind="ExternalInput")
    W1AUGd = nc.dram_tensor("W1AUG", [IN_DIM, HID + 2 * H1], BF16,
                            kind="ExternalInput")
    W2AUGd = nc.dram_tensor("W2AUG", [HID, 2 + OUT_DIM], BF16,
                            kind="ExternalInput")
    B1Rd = nc.dram_tensor("B1R", [P, HID], F32, kind="ExternalInput")
    B2Rd = nc.dram_tensor("B2R", [P, OUT_DIM], F32, kind="ExternalInput")
    IOTABd = nc.dram_tensor("IOTAB", [P, KMAX * P], BF16,
                            kind="ExternalInput")
    IDENTd = nc.dram_tensor("IDENT", [P, P], BF16, kind="ExternalInput")
    ISRCd = nc.dram_tensor("ISRC", [P, NCHTOT], I32, kind="ExternalInput")
    IDST0d = nc.dram_tensor("IDST0", [P, BPC], I32, kind="ExternalInput")
    DLOCd = nc.dram_tensor("DLOC", [P, NCHTOT], BF16, kind="ExternalInput")
    OUTd = nc.dram_tensor("OUT", [BPC * P, OUT_DIM], F32,
                          kind="ExternalOutput")

    # ---------------- internal DRAM ----------------
    T1Ld = nc.dram_tensor("T1L", [BPC * P, T1W], BF16)
    DSTTLd = nc.dram_tensor("DSTTL", [BPC * P, DSTW], BF16)
    T1d = nc.dram_tensor("T1", [NPAD, T1W], BF16, addr_space="Shared")
    DSTTd = nc.dram_tensor("DSTT", [NPAD, DSTW], BF16, addr_space="Shared")
    DSTT2d = nc.dram_tensor("DSTT2", [BPC * P, 4], F32)
    T2Ld = nc.dram_tensor("T2L", [BPC * P, T2W], F32)
    T2d = nc.dram_tensor("T2", [NPAD, T2W], F32, addr_space="Shared")

    with tile.TileContext(nc) as tc:
        with tc.tile_pool(name="consts", bufs=1) as cp:
            W1AUG_sb = cp.tile([IN_DIM, HID + 2 * H1], BF16)
            nc.sync.dma_start(out=W1AUG_sb[:], in_=W1AUGd[:, :])
            W2AUG_sb = cp.tile([HID, 2 + OUT_DIM], BF16)
            nc.sync.dma_start(out=W2AUG_sb[:], in_=W2AUGd[:, :])
            B1R_sb = cp.tile([P, HID], F32)
            nc.sync.dma_start(out=B1R_sb[:], in_=B1Rd[:, :])
            B2R_sb = cp.tile([P, OUT_DIM], F32)
            nc.sync.dma_start(out=B2R_sb[:], in_=B2Rd[:, :])
            IOTAB_sb = cp.tile([P, KMAX * P], BF16)
            nc.sync.dma_start(out=IOTAB_sb[:], in_=IOTABd[:, :])
            IDENT_sb = cp.tile([P, P], BF16)
            nc.sync.dma_start(out=IDENT_sb[:], in_=IDENTd[:, :])
            DLOC_sb = cp.tile([P, NCHTOT], BF16)
            nc.sync.dma_start(out=DLOC_sb[:], in_=DLOCd[:, :])
            ISRC_sb = cp.tile([P, NCHTOT], I32)
            nc.sync.dma_start(out=ISRC_sb[:], in_=ISRCd[:, :])
            IDST0_sb = cp.tile([P, BPC], I32)
            nc.sync.dma_start(out=IDST0_sb[:], in_=IDST0d[:, :])

            # ================= Phase A: T1 / DSTT generation ================
            with tc.tile_pool(name="pa", bufs=3) as pa, \
                 tc.tile_pool(name="papsum", bufs=2, space="PSUM") as pap:
                for tb in range(BPC // TPB):
                    c0 = tb * TPB * P
                    xt = pa.tile([IN_DIM, TPB * P], BF16, tag="xt")
                    nc.sync.dma_start(out=xt[:], in_=XTB[:, c0:c0 + TPB * P])
                    # block i at a 2KB-aligned 512-f32 stride so each matmul
                    # output sits in one PSUM bank
                    hal = pap.tile([P, TPB * 512], F32, tag="hal")
                    hal4 = hal[:].rearrange("p (t c) -> p t c", c=512)
                    for i in range(TPB):
                        nc.tensor.matmul(
                            hal[:, i * 512:i * 512 + 144],
                            lhsT=xt[:, i * P:(i + 1) * P], rhs=W1AUG_sb[:],
                            start=True, stop=True)
                    t1 = pa.tile([P, TPB, T1W], BF16, tag="t1")
                    nc.vector.tensor_copy(out=t1[:, :, 8:136],
                                          in_=hal4[:, :, 0:128])
                    nc.vector.tensor_copy(out=t1[:, :, 0:8],
                                          in_=hal4[:, :, 136:144])
                    dstt = pa.tile([P, TPB, DSTW], BF16, tag="dstt")
                    nc.vector.tensor_copy(out=dstt[:, :, 0:8],
                                          in_=hal4[:, :, 128:136])
                    nc.sync.dma_start(
                        out=T1Ld[c0:c0 + TPB * P, :].rearrange(
                            "(t p) e -> p t e", p=P),
                        in_=t1[:])
                    nc.scalar.dma_start(
                        out=DSTTLd[c0:c0 + TPB * P, :].rearrange(
                            "(t p) e -> p t e", p=P),
                        in_=dstt[:])

            # AllGather the phase-A table shards
            nc.gpsimd.collective_compute(
                "AllGather", OP.bypass,
                replica_groups=[list(range(n_cores))],
                ins=[T1Ld[:, :].opt()],
                outs=[T1d[:, :].opt()],
            )
            nc.gpsimd.collective_compute(
                "AllGather", OP.bypass,
                replica_groups=[list(range(n_cores))],
                ins=[DSTTLd[:, :].opt()],
                outs=[DSTTd[:, :].opt()],
            )

            _phase_barrier(tc, nc)

            # ============ Phase B/C: edge processing (shared shape) =========
            def edge_phase(layer):
                sdt = BF16 if layer == 1 else F32
                srcw = T1W if layer == 1 else T2W
                srcwp = 144 if layer == 1 else 48   # padded tile stride
                dstw = DSTW if layer == 1 else 4
                dstwp = 16 if layer == 1 else 8     # padded tile stride
                accw = ACC1W if layer == 1 else ACC2W
                nhd = H1 if layer == 1 else 1
                fdim = HID if layer == 1 else OUT_DIM
                srcT = T1d if layer == 1 else T2d
                tg = f"L{layer}"

                with tc.tile_pool(name=f"pg{layer}", bufs=4) as pg, \
                     tc.tile_pool(name=f"pm{layer}", bufs=2) as pm, \
                     tc.tile_pool(name=f"pe{layer}", bufs=2) as pe, \
                     tc.tile_pool(name=f"pp{layer}", bufs=1,
                                  space="PSUM") as pp, \
                     tc.tile_pool(name=f"ppm{layer}", bufs=2,
                                  space="PSUM") as ppm, \
                     tc.tile_pool(name=f"ppq{layer}", bufs=1,
                                  space="PSUM") as ppq, \
                     tc.tile_pool(name=f"pq{layer}", bufs=1,
                                  space="PSUM") as pq:
                    for g in range(NGRP):
                        ch0g = int(grp_ch0[g])
                        nchg = int(NCHG[g])
                        # chunk -> (slot-in-group, chunk-in-slot)
                        sl_of = []
                        for s7 in range(G):
                            for c in range(int(NCHS[g, s7])):
                                sl_of.append((s7, c))
                        # per-slot acc regions at a 1KB stride (two slots
                        # per bank; safe: a slot's accumulation group fully
                        # precedes its bank-sibling's start, and start=True
                        # only clears accumulate bits, not data)
                        accT = pp.tile([P, G * 256], F32, tag="acc",
                                       name=f"acc{layer}_{g}")
                        # per-group adst tiles (replaces per-chunk dst
                        # gathers: 2x fewer indirect calls)
                        if layer == 1:
                            adstg = pg.tile([P, G, DSTW], BF16, tag="adstg")
                            for s7 in range(G):
                                nc.gpsimd.indirect_dma_start(
                                    out=adstg[:, s7, :], out_offset=None,
                                    in_=DSTTd[:, :],
                                    in_offset=bass.IndirectOffsetOnAxis(
                                        ap=IDST0_sb[:, g * G + s7:
                                                    g * G + s7 + 1],
                                        axis=0))
                        else:
                            # SWDGE cast f32 -> bf16 during the load (the PE
                            # rejects mixed f32/bf16 matmul operands)
                            adstg = pg.tile([P, G, 4], BF16, tag="adstg2")
                            nc.gpsimd.dma_start(
                                out=adstg[:],
                                in_=DSTT2d[g * GROWS:(g + 1) * GROWS, :]
                                    .rearrange("(t p) e -> p t e", p=P))
                        for (a, b) in _ranges(nchg, NRQ):
                            k = b - a
                            c0 = ch0g + a
                            # one [128,1]-offset indirect per chunk (the
                            # multi-offset form mispairs offsets on HW)
                            gt = pg.tile([P, KMAX, srcwp], sdt,
                                         tag=f"gt{tg}")
                            for j in range(k):
                                nc.gpsimd.indirect_dma_start(
                                    out=gt[:, j, 0:srcw], out_offset=None,
                                    in_=srcT[:, :],
                                    in_offset=bass.IndirectOffsetOnAxis(
                                        ap=ISRC_sb[:, c0 + j:c0 + j + 1],
                                        axis=0))

                            # ---- batched edge compute for this range ----
                            mask = pm.tile([P, KMAX, P], BF16,
                                           tag=f"mask{tg}")
                            nc.vector.tensor_tensor(
                                out=mask[:, 0:k, :],
                                in0=IOTAB_sb[:, 0:k * P].rearrange(
                                    "p (n d) -> p n d", d=P),
                                in1=DLOC_sb[:, c0:c0 + k]
                                    .unsqueeze(2).to_broadcast([P, k, P]),
                                op=OP.is_equal)
                            # maskT (PE transposes, batched PSUM->SBUF
                            # copies), then per-edge adst = maskT.T @
                            # adst_block on the tensor engine
                            mts = pm.tile([P, KMAX, P], BF16,
                                          tag=f"mts{tg}")
                            for j0 in range(0, k, 8):
                                jn = min(8, k - j0)
                                mt_ps = ppm.tile([P, 8, P], BF16,
                                                 tag="mtps",
                                                 name=f"mtps{layer}_{g}_"
                                                      f"{c0}_{j0}")
                                for j in range(j0, j0 + jn):
                                    nc.tensor.transpose(
                                        mt_ps[:, j - j0, :],
                                        mask[:, j, :], IDENT_sb[:])
                                nc.vector.tensor_copy(
                                    out=mts[:, j0:j0 + jn, :],
                                    in_=mt_ps[:, 0:jn, :])
                            adps = ppq.tile([P, 512], F32, tag="adps",
                                            name=f"adps{layer}_{g}_{c0}")
                            apv = adps[:, 0:k * nhd].rearrange(
                                "p (n h) -> p n h", h=nhd)
                            for j in range(k):
                                s7j = sl_of[a + j][0]
                                nc.tensor.matmul(
                                    apv[:, j, :],
                                    lhsT=mts[:, j, :],
                                    rhs=adstg[:, s7j, 0:nhd],
                                    start=True, stop=True,
                                    skip_group_check=True)
                            lg = pm.tile([P, KMAX, nhd], F32, tag=f"lg{tg}")
                            nc.vector.tensor_tensor(
                                out=lg[:, 0:k, :],
                                in0=gt[:, 0:k, 0:nhd],
                                in1=apv[:, 0:k, :], op=OP.add)
                            e1 = pm.tile([P, KMAX, nhd], F32, tag=f"e1{tg}")
                            nc.scalar.activation(out=e1[:, 0:k, :],
                                                 in_=lg[:, 0:k, :],
                                                 func=AF.Exp)
                            e2 = pm.tile([P, KMAX, nhd], F32, tag=f"e2{tg}")
                            nc.scalar.activation(out=e2[:, 0:k, :],
                                                 in_=lg[:, 0:k, :],
                                                 func=AF.Exp, scale=NEG_SLOPE)
                            msg = pm.tile([P, KMAX, accw], BF16,
                                          tag=f"msg{tg}")
                            nc.vector.tensor_tensor(
                                out=msg[:, 0:k, fdim:accw],
                                in0=e1[:, 0:k, :], in1=e2[:, 0:k, :],
                                op=OP.max)
                            if layer == 1:
                                m4 = msg[:, 0:k, 0:fdim].rearrange(
                                    "p n (h c) -> p n h c", c=C1)
                                h4 = gt[:, 0:k, 8:136].rearrange(
                                    "p n (h c) -> p n h c", c=C1)
                                x4 = msg[:, 0:k, fdim:accw].unsqueeze(3) \
                                    .to_broadcast([P, k, H1, C1])
                                nc.vector.tensor_tensor(
                                    out=m4, in0=h4, in1=x4, op=OP.mult)
                            else:
                                nc.vector.tensor_tensor(
                                    out=msg[:, 0:k, 0:fdim],
                                    in0=gt[:, 0:k, 1:1 + OUT_DIM],
                                    in1=msg[:, 0:k, fdim:accw]
                                        .to_broadcast([P, k, OUT_DIM]),
                                    op=OP.mult)

                            # ---- scatter-accumulate per chunk ----
                            for j in range(a, b):
                                s7, c = sl_of[j]
                                nc.tensor.matmul(
                                    accT[:, s7 * 256:s7 * 256 + accw],
                                    lhsT=mask[:, j - a, :],
                                    rhs=msg[:, j - a, :],
                                    start=(c == 0),
                                    stop=(c == int(NCHS[g, s7]) - 1),
                                    skip_group_check=True)

                        # ---- group epilogue ----
                        accsb = pe.tile([P, G, accw], F32, tag=f"accsb{tg}")
                        for s7 in range(G):
                            nc.vector.tensor_copy(
                                out=accsb[:, s7, :],
                                in_=accT[:, s7 * 256:s7 * 256 + accw])
                        if layer == 1:
                            dinv = pe.tile([P, G, H1], F32, tag="dinv")
                            nc.vector.tensor_scalar(
                                out=dinv[:], in0=accsb[:, :, HID:ACC1W],
                                scalar1=EPS, scalar2=DENOM_FLOOR,
                                op0=OP.add, op1=OP.max)
                            nc.vector.reciprocal(out=dinv[:], in_=dinv[:])
                            h1 = pe.tile([P, G, HID], F32, tag="h1")
                            a4 = accsb[:, :, 0:HID].rearrange(
                                "p g (h c) -> p g h c", c=C1)
                            dv4 = dinv[:].unsqueeze(3).to_broadcast(
                                [P, G, H1, C1])
                            h14 = h1[:].rearrange(
                                "p g (h c) -> p g h c", c=C1)
                            nc.vector.tensor_tensor(
                                out=h14, in0=a4, in1=dv4, op=OP.mult)
                            nc.vector.tensor_tensor(
                                out=h1[:], in0=h1[:],
                                in1=B1R_sb[:].unsqueeze(1)
                                    .to_broadcast([P, G, HID]),
                                op=OP.add)
                            # ELU(x) = max(x,0) + min(exp(x)-1, 0)
                            ex = pe.tile([P, G, HID], F32, tag="ex")
                            nc.scalar.activation(out=ex[:], in_=h1[:],
                                                 func=AF.Exp)
                            nc.vector.tensor_scalar(
                                out=ex[:], in0=ex[:], scalar1=-1.0,
                                scalar2=0.0, op0=OP.add, op1=OP.min)
                            nc.vector.tensor_scalar(
                                out=h1[:], in0=h1[:], scalar1=0.0,
                                scalar2=None, op0=OP.max)
                            h1e = pe.tile([P, G, HID], BF16, tag="h1e")
                            nc.vector.tensor_tensor(out=h1e[:], in0=h1[:],
                                                    in1=ex[:], op=OP.add)
                            for s7 in range(G):
                                s = g * G + s7
                                epi = pq.tile([P, 1024], BF16, tag="epi",
                                              name=f"epi_{g}_{s7}")
                                h1T_ps = epi[:, 0:HID]
                                nc.tensor.transpose(h1T_ps,
                                                    h1e[:, s7, :],
                                                    IDENT_sb[:])
                                h1T = pe.tile([P, HID], BF16, tag="h1Ts")
                                nc.vector.tensor_copy(out=h1T[:],
                                                      in_=h1T_ps)
                                hg2x = epi[:, 256:256 + 2 * (2 + OUT_DIM)]
                                hg2 = hg2x.bitcast(F32)
                                nc.tensor.matmul(hg2, lhsT=h1T[:],
                                                 rhs=W2AUG_sb[:],
                                                 start=True, stop=True)
                                t2row = pe.tile([P, T2W], F32, tag="t2r")
                                nc.vector.tensor_copy(
                                    out=t2row[:],
                                    in_=hg2[:, 1:2 + OUT_DIM])
                                d2row = pe.tile([P, 4], F32, tag="d2r")
                                nc.vector.tensor_copy(out=d2row[:, 0:1],
                                                      in_=hg2[:, 0:1])
                                nc.scalar.dma_start(
                                    out=T2Ld[s * P:(s + 1) * P, :],
                                    in_=t2row[:])
                                nc.scalar.dma_start(
                                    out=DSTT2d[s * P:(s + 1) * P, :],
                                    in_=d2row[:])
                        else:
                            dinv2 = pe.tile([P, G, 1], F32, tag="dinv2")
                            nc.vector.tensor_scalar(
                                out=dinv2[:], in0=accsb[:, :, OUT_DIM:ACC2W],
                                scalar1=EPS, scalar2=DENOM_FLOOR,
                                op0=OP.add, op1=OP.max)
                            nc.vector.reciprocal(out=dinv2[:], in_=dinv2[:])
                            o = pe.tile([P, G, OUT_DIM], F32, tag="o")
                            nc.vector.tensor_tensor(
                                out=o[:], in0=accsb[:, :, 0:OUT_DIM],
                                in1=dinv2[:].to_broadcast([P, G, OUT_DIM]),
                                op=OP.mult)
                            nc.vector.tensor_tensor(
                                out=o[:], in0=o[:],
                                in1=B2R_sb[:].unsqueeze(1)
                                    .to_broadcast([P, G, OUT_DIM]),
                                op=OP.add)
                            # log_softmax = (o - m) - ln(sum(exp(o - m)))
                            nm = pe.tile([P, G, 1], F32, tag="nm")
                            nc.vector.tensor_reduce(
                                out=nm[:], in_=o[:],
                                axis=mybir.AxisListType.X,
                                op=OP.max, negate=True)
                            osh = pe.tile([P, G, OUT_DIM], F32, tag="osh")
                            nc.vector.tensor_tensor(
                                out=osh[:], in0=o[:],
                                in1=nm[:].to_broadcast([P, G, OUT_DIM]),
                                op=OP.add)
                            e2t = pe.tile([P, G, OUT_DIM], F32, tag="e2t")
                            nc.scalar.activation(out=e2t[:], in_=osh[:],
                                                 func=AF.Exp)
                            s2 = pe.tile([P, G, 1], F32, tag="s2")
                            nc.vector.tensor_reduce(
                                out=s2[:], in_=e2t[:],
                                axis=mybir.AxisListType.X, op=OP.add)
                            ls = pe.tile([P, G, 1], F32, tag="ls")
                            nc.scalar.activation(out=ls[:], in_=s2[:],
                                                 func=AF.Ln)
                            ot = pe.tile([P, G, OUT_DIM], F32, tag="ot")
                            nc.vector.tensor_tensor(
                                out=ot[:], in0=osh[:],
                                in1=ls[:].to_broadcast([P, G, OUT_DIM]),
                                op=OP.subtract)
                            nc.sync.dma_start(
                                out=OUTd[g * GROWS:(g + 1) * GROWS, :]
                                    .rearrange("(t p) e -> p t e", p=P),
                                in_=ot[:])

            edge_phase(1)

            # ============= AllGather of T2 shards ===============
            nc.gpsimd.collective_compute(
                "AllGather", OP.bypass,
                replica_groups=[list(range(n_cores))],
                ins=[T2Ld[:, :].opt()],
                outs=[T2d[:, :].opt()],
            )
            _phase_barrier(tc, nc)

            edge_phase(2)

    return nc


# ----------------------------------------------------------------------------
# host-side preprocessing (index/layout work)
# ----------------------------------------------------------------------------

def preprocess_graph(src, dst, n_nodes):
    """Assign nodes to in-degree-balanced blocks of 128."""
    deg = np.bincount(dst, minlength=n_nodes)
    order = np.argsort(-deg, kind="stable")
    r = np.arange(n_nodes)
    rounds, posr = r // NBLK, r % NBLK
    binr = np.where(rounds % 2 == 0, posr, NBLK - 1 - posr)
    blk_of_node = np.empty(n_nodes, np.int64)
    blk_of_node[order] = binr
    cnt = np.bincount(blk_of_node, minlength=NBLK)
    assert cnt.max() <= P, f"block overfull: {cnt.max()}"
    node_sorted = np.argsort(blk_of_node, kind="stable")
    starts = np.concatenate([[0], np.cumsum(cnt)[:-1]])
    slot_sorted = np.arange(n_nodes) - np.repeat(starts, cnt)
    slot_of_node = np.empty(n_nodes, np.int64)
    slot_of_node[node_sorted] = slot_sorted
    pos_of_node = blk_of_node * P + slot_of_node
    return pos_of_node, blk_of_node, slot_of_node


def build_edge_tables(src, dst, pos, blk, slot):
    """Uniform chunk grid + per-core offset/dloc tables."""
    dblk = blk[dst]
    core = dblk // BPC
    dslot = dblk % BPC
    spos = pos[src]

    key = core * BPC + dslot
    order = np.argsort(key, kind="stable")
    cnts = np.bincount(key, minlength=N_CORES * BPC)
    ch = np.ceil(cnts / P).astype(np.int64).reshape(N_CORES, BPC)
    cm = np.maximum(ch.max(axis=0), 1)              # [BPC]

    NCHTOT = int(cm.sum())
    starts = np.concatenate([[0], np.cumsum(cnts)[:-1]])
    chcol_of_slot = np.concatenate([[0], np.cumsum(cm)[:-1]])

    dst_sl = slot[dst].astype(np.int32)

    per_core = []
    for k in range(N_CORES):
        isrc = np.zeros((P, NCHTOT), np.int32)
        dloc = np.full((P, NCHTOT), PADLOC, np.float32)
        for s in range(BPC):
            ki = k * BPC + s
            n = cnts[ki]
            e = order[starts[ki]:starts[ki] + n]
            chcol = chcol_of_slot[s]
            cols = chcol + np.arange(n) // P
            rows = np.arange(n) % P
            isrc[rows, cols] = spos[e]
            dloc[rows, cols] = dst_sl[e]
        # identity row offsets of each owned block (for per-group adst loads)
        idst0 = ((k * BPC + np.arange(BPC))[None, :] * P
                 + np.arange(P)[:, None]).astype(np.int32)
        per_core.append({
            "ISRC": isrc,
            "IDST0": idst0,
            "DLOC": dloc.astype(ml_dtypes.bfloat16),
        })
    return cm, per_core


def build_inputs(x, edge_index, W1, a_src1, a_dst1, b1, W2, a_src2, a_dst2,
                 b2, n_cores):
    src = np.asarray(edge_index[0], dtype=np.int64)
    dst = np.asarray(edge_index[1], dtype=np.int64)
    pos, blk, slot = preprocess_graph(src, dst, N_NODES)
    cm, per_core = build_edge_tables(src, dst, pos, blk, slot)
    NCHG = cm.reshape(NGRP, G).sum(axis=1)
    KMAX = max(b - a for g in range(NGRP)
               for (a, b) in _ranges(int(NCHG[g]), NRQ))

    x = np.asarray(x, np.float32)
    XTa = np.zeros((IN_DIM, NPAD), np.float32)
    XTa[:, pos] = x.T

    W1 = np.asarray(W1, np.float32)
    W2 = np.asarray(W2, np.float32)
    a_src1 = np.asarray(a_src1, np.float32)
    a_dst1 = np.asarray(a_dst1, np.float32)
    a_src2 = np.asarray(a_src2, np.float32)
    a_dst2 = np.asarray(a_dst2, np.float32)
    b1 = np.asarray(b1, np.float32)
    b2 = np.asarray(b2, np.float32)

    # A1BD columns: [adst1(8) | asrc1(8)] per-head block-diagonal
    A1BD = np.zeros((HID, 2 * H1), np.float32)
    for h in range(H1):
        A1BD[h * C1:(h + 1) * C1, h] = a_dst1[h]
        A1BD[h * C1:(h + 1) * C1, H1 + h] = a_src1[h]
    W1AUG = np.concatenate([W1, W1 @ A1BD], axis=1)

    A2T = np.stack([a_dst2[0], a_src2[0]], axis=1)     # [OUT_DIM, 2]
    W2AUG = np.concatenate([W2 @ A2T, W2], axis=1)     # [HID, 2+OUT_DIM]

    iota = np.broadcast_to(np.arange(P, dtype=np.float32), (P, P))
    iotab = np.tile(iota, (1, KMAX))

    XTb = XTa.astype(ml_dtypes.bfloat16)
    common = {
        "W1AUG": W1AUG.astype(ml_dtypes.bfloat16),
        "W2AUG": W2AUG.astype(ml_dtypes.bfloat16),
        "B1R": np.ascontiguousarray(np.broadcast_to(b1, (P, HID))),
        "B2R": np.ascontiguousarray(np.broadcast_to(b2, (P, OUT_DIM))),
        "IOTAB": np.ascontiguousarray(iotab).astype(ml_dtypes.bfloat16),
        "IDENT": np.eye(P, dtype=ml_dtypes.bfloat16),
    }
    in_maps = []
    for k, pc in enumerate(per_core):
        m = dict(common, **pc)
        m["XTB"] = np.ascontiguousarray(
            XTb[:, k * BPC * P:(k + 1) * BPC * P])
        in_maps.append(m)
    return in_maps, pos, cm


# ----------------------------------------------------------------------------
# entry point
# ----------------------------------------------------------------------------

_prog_cache = {}
last_results = None


def _get_program(cm, n_cores):
    key = (cm.tobytes(), n_cores)
    if key not in _prog_cache:
        nc = build_program(cm, n_cores)
        _split_excess_waits(nc)
        _prog_cache[key] = nc
    return _prog_cache[key]


def run(inputs, n_cores=N_CORES, trace=False):
    global last_results
    in_maps, pos, cm = build_inputs(n_cores=n_cores, **inputs)
    nc = _get_program(cm, n_cores)
    kwargs = {}
    if trace:
        kwargs = dict(trace=True, trace_cores=[0])
    res = run_bass_kernel_spmd(
        nc, in_maps, core_ids=list(range(n_cores)), **kwargs)
    last_results = res
    out_all = np.concatenate([r["OUT"] for r in res.results], axis=0)
    return np.ascontiguousarray(out_all[pos].astype(np.float32))


def kernel(**inputs):
    return run(inputs)
